# revision 12
# baseline (speedup 1.0000x reference)
"""Trainium2 Bass kernel for CapsuleParall dynamic routing.

Math (per (b, n) pair, u_hat[i,o] = u[i] * W[n][i,o]):
    s_1[o] = sum_i u_hat[i,o] * c0[i,o]
    v_k    = squash(s_k + bias)           (squash over o)
    V_k    = v_1 + ... + v_k              (cumulative; b == u_hat * V)
    c_k    = softmax_o(u_hat[i,o] * V_k[o])
    s_{k+1}[o] = sum_i u_hat[i,o] * c_k[i,o]
    out    = squash(s_routings + bias)

Key identity used on-chip: softmax is computed unnormalized,
    e[i,o] = exp(u_hat[i,o] * V[o]),   Z[i] = sum_o e[i,o]
    s[o] = sum_i (W[i,o]*e[i,o]) * (u[i]/Z[i])
so the PE matmul (lhsT = W.e chunk, rhs = u/Z column) applies both the
u factor and the softmax normalization during the i-contraction.

Sharding: data-parallel over batch B across 8 cores (4 batches/core).
"""

import sys

sys.path.insert(0, "/opt/trn_rl_repo")

from contextlib import ExitStack

import numpy as np

import concourse.bass as bass
import concourse.bacc as bacc
import concourse.mybir as mybir
import concourse.tile as tile
from concourse import masks
from concourse.bass_utils import run_bass_kernel_spmd

F32 = mybir.dt.float32
EPS = 1e-5
N_CORES = 8


def _build(B_core, NUM, IN_F, OUT_F, routings, c00, uniform_c0):
    """Build the per-core Bass module. Returns (nc, names)."""
    P = 128
    assert IN_F % P == 0
    T = IN_F // P                      # 9 i-chunks
    PAIRS = B_core * NUM               # 64 (b, n) pairs per core
    # squash groups must start at partition 0/32/64/96 (HW AP restriction)
    GP = 32 if (PAIRS % 32 == 0 and PAIRS > 32) else PAIRS
    G = PAIRS // GP
    mult = mybir.AluOpType.mult
    add = mybir.AluOpType.add

    nc = bacc.Bacc("TRN2", target_bir_lowering=False, debug=False)

    u_dram = nc.dram_tensor("u", [B_core, NUM, IN_F], F32, kind="ExternalInput")
    w_dram = nc.dram_tensor("w", [NUM, IN_F, OUT_F], F32, kind="ExternalInput")
    b_dram = nc.dram_tensor("bias", [NUM, OUT_F], F32, kind="ExternalInput")
    if not uniform_c0:
        c0_dram = nc.dram_tensor("c0", [IN_F, OUT_F], F32, kind="ExternalInput")
    out_dram = nc.dram_tensor("out", [B_core, NUM, OUT_F], F32, kind="ExternalOutput")

    def bcast_mid(ap2d, n):
        # [P, F] -> [P, n, F] with the middle dim broadcast (stride 0)
        return bass.AP(ap2d.tensor, ap2d.offset, [ap2d.ap[0], [0, n], ap2d.ap[1]])

    with tile.TileContext(nc) as tc, ExitStack() as ctx:
        const = ctx.enter_context(tc.tile_pool(name="const", bufs=1))
        work = ctx.enter_context(tc.tile_pool(name="work", bufs=3))
        small = ctx.enter_context(tc.tile_pool(name="small", bufs=4))
        sall_pool = ctx.enter_context(tc.tile_pool(name="sall", bufs=2))
        sq_pool = ctx.enter_context(tc.tile_pool(name="sq", bufs=4))
        psum_s = ctx.enter_context(
            tc.tile_pool(name="psum_s", bufs=3, space=bass.MemorySpace.PSUM)
        )
        psum_vb = ctx.enter_context(
            tc.tile_pool(name="psum_vb", bufs=2, space=bass.MemorySpace.PSUM)
        )
        psum_tr = ctx.enter_context(
            tc.tile_pool(name="psum_tr", bufs=2, space=bass.MemorySpace.PSUM)
        )

        # ---- resident tensors ----
        W_sb = const.tile([P, NUM, T, OUT_F], F32)       # W[n][i,o], i = t*128+p
        u_nat = const.tile([PAIRS, IN_F], F32)           # natural row layout
        u_sb = const.tile([P, T, PAIRS], F32)            # u columns (i on partitions)
        uc_sb = const.tile([P, T, PAIRS], F32)           # u * c00 (uniform-c0 path)
        bias_all = const.tile([PAIRS, OUT_F], F32)
        ident = const.tile([P, P], F32)
        ones_row = const.tile([1, P], F32)
        if not uniform_c0:
            c0_sb = const.tile([P, T, OUT_F], F32)

        # ---- loads ----
        w_ap = w_dram.ap()
        # src AP dims: [p(128), n, t, o] in elements of w_dram [NUM, IN_F, OUT_F]
        w_src = bass.AP(
            w_ap.tensor,
            w_ap.offset,
            [[OUT_F, P], [IN_F * OUT_F, NUM], [P * OUT_F, T], [1, OUT_F]],
        )
        nc.sync.dma_start(W_sb[:, :, :, :], w_src)
        nc.sync.dma_start(u_nat[:, :], u_dram.ap().rearrange("b n i -> (b n) i"))
        for b in range(B_core):
            nc.sync.dma_start(bias_all[b * NUM : (b + 1) * NUM, :], b_dram.ap())
        if not uniform_c0:
            c_ap = c0_dram.ap()
            c_src = bass.AP(
                c_ap.tensor, c_ap.offset, [[OUT_F, P], [P * OUT_F, T], [1, OUT_F]]
            )
            nc.sync.dma_start(c0_sb[:, :, :], c_src)
        masks.make_identity(nc, ident[:, :])
        nc.vector.memset(ones_row[:, :], 1.0)

        # u_nat [PAIRS, IN_F] -> u_sb [P, T, PAIRS] via PE transposes per chunk
        for t in range(T):
            tr = psum_tr.tile([P, PAIRS], F32, tag="tr")
            nc.tensor.transpose(
                tr[:, :], u_nat[:, t * P : (t + 1) * P], ident[:PAIRS, :PAIRS]
            )
            nc.vector.tensor_copy(u_sb[:, t, :], tr[:, :])
        nc.vector.tensor_scalar_mul(uc_sb[:, :, :], u_sb[:, :, :], float(c00))

        # ---- phase 1: s_1 for every pair ----
        s_allT = sall_pool.tile([P, PAIRS], F32, tag="sall")
        for p in range(PAIRS):
            n = p % NUM
            s_col = psum_s.tile([OUT_F, 1], F32, tag="scol")
            for t in range(T):
                if uniform_c0:
                    nc.tensor.matmul(
                        s_col[:, :],
                        W_sb[:, n, t, :],
                        uc_sb[:, t, p : p + 1],
                        start=(t == 0),
                        stop=(t == T - 1),
                    )
                else:
                    wc = work.tile([P, OUT_F], F32, tag="wc0")
                    nc.vector.tensor_tensor(
                        wc[:, :], W_sb[:, n, t, :], c0_sb[:, t, :], op=mult
                    )
                    nc.tensor.matmul(
                        s_col[:, :],
                        wc[:, :],
                        u_sb[:, t, p : p + 1],
                        start=(t == 0),
                        stop=(t == T - 1),
                    )
            nc.vector.tensor_copy(s_allT[:, p : p + 1], s_col[:, :])

        # ---- squash (batched over a group of GP pairs) ----
        def squash_group(s_tile, g, V_prev, is_final):
            r0, r1 = g * GP, (g + 1) * GP
            tr = psum_tr.tile([GP, OUT_F], F32, tag="tr")
            nc.tensor.transpose(tr[:, :], s_tile[:, r0:r1], ident[:, :])
            sb = sq_pool.tile([GP, OUT_F], F32, tag="sb")
            nc.vector.tensor_tensor(sb[:, :], tr[:, :], bias_all[r0:r1, :], op=add)
            sqs = sq_pool.tile([GP, OUT_F], F32, tag="sqs")
            n2 = sq_pool.tile([GP, 1], F32, tag="n2")
            nc.scalar.activation(
                sqs[:, :],
                sb[:, :],
                mybir.ActivationFunctionType.Square,
                accum_out=n2[:, :],
            )
            rt = sq_pool.tile([GP, 1], F32, tag="rt")
            nc.scalar.activation(rt[:, :], n2[:, :], mybir.ActivationFunctionType.Sqrt)
            d1 = sq_pool.tile([GP, 1], F32, tag="d1")
            nc.vector.tensor_scalar_add(d1[:, :], n2[:, :], 1.0)
            d2 = sq_pool.tile([GP, 1], F32, tag="d2")
            nc.vector.tensor_scalar_add(d2[:, :], rt[:, :], EPS)
            den = sq_pool.tile([GP, 1], F32, tag="den")
            nc.vector.tensor_tensor(den[:, :], d1[:, :], d2[:, :], op=mult)
            rden = sq_pool.tile([GP, 1], F32, tag="rden")
            nc.vector.reciprocal(rden[:, :], den[:, :])
            coef = sq_pool.tile([GP, 1], F32, tag="coef")
            nc.vector.tensor_tensor(coef[:, :], n2[:, :], rden[:, :], op=mult)
            v = sq_pool.tile([GP, OUT_F], F32, tag="v")
            nc.vector.tensor_scalar_mul(v[:, :], sb[:, :], coef[:, 0:1])
            if is_final:
                out_rows = out_dram.ap().rearrange("b n o -> (b n) o")
                nc.sync.dma_start(out_rows[r0:r1, :], v[:, :])
                return None
            if V_prev is None:
                V_new = v
            else:
                V_new = sq_pool.tile([GP, OUT_F], F32, tag="V")
                nc.vector.tensor_tensor(V_new[:, :], V_prev[:, :], v[:, :], op=add)
            # flat row copy at partition 0 so PE outer-products can read it
            V_flat = sq_pool.tile([1, GP * OUT_F], F32, tag="Vflat")
            nc.sync.dma_start(V_flat[0:1, :], V_new[:, :])
            return (V_new, V_flat)

        # ---- routing iterations ----
        V_cur = [None] * G
        s_cur = s_allT
        for k in range(2, routings + 1):
            s_next = sall_pool.tile([P, PAIRS], F32, tag="sall")
            for g in range(G):
                prev = V_cur[g][0] if V_cur[g] is not None else None
                V_cur[g] = squash_group(s_cur, g, prev, is_final=False)
            for p in range(PAIRS):
                g, pl = p // GP, p % GP
                n = p % NUM
                # broadcast V row -> [P, OUT_F] (outer product with ones row)
                vb = psum_vb.tile([P, OUT_F], F32, tag="vb")
                nc.tensor.matmul(
                    vb[:, :],
                    ones_row[:, :],
                    V_cur[g][1][0:1, pl * OUT_F : (pl + 1) * OUT_F],
                    start=True,
                    stop=True,
                )
                # u_hat = W * u (per-partition column scalar), chunk by chunk
                u_hat = work.tile([P, T, OUT_F], F32, tag="uhat")
                for t in range(T):
                    nc.vector.tensor_scalar_mul(
                        u_hat[:, t, :], W_sb[:, n, t, :], u_sb[:, t, p : p + 1]
                    )
                tt = work.tile([P, T, OUT_F], F32, tag="tmp")
                nc.vector.tensor_tensor(
                    tt[:, :, :], u_hat[:, :, :], bcast_mid(vb[:, :], T), op=mult
                )
                et = work.tile([P, T, OUT_F], F32, tag="e")
                nc.scalar.activation(
                    et[:, :, :], tt[:, :, :], mybir.ActivationFunctionType.Exp
                )
                Z = small.tile([P, T], F32, tag="Z")
                nc.vector.tensor_reduce(
                    Z[:, :], et[:, :, :], axis=mybir.AxisListType.X, op=add
                )
                wr = small.tile([P, T], F32, tag="wr")
                nc.vector.reciprocal(wr[:, :], Z[:, :])
                uw = small.tile([P, T], F32, tag="uw")
                nc.vector.tensor_tensor(uw[:, :], wr[:, :], u_sb[:, :, p], op=mult)
                ft = work.tile([P, T, OUT_F], F32, tag="tmp")
                nc.vector.tensor_tensor(
                    ft[:, :, :], et[:, :, :], W_sb[:, n, :, :], op=mult
                )
                s_col = psum_s.tile([OUT_F, 1], F32, tag="scol")
                for t in range(T):
                    nc.tensor.matmul(
                        s_col[:, :],
                        ft[:, t, :],
                        uw[:, t : t + 1],
                        start=(t == 0),
                        stop=(t == T - 1),
                    )
                nc.vector.tensor_copy(s_next[:, p : p + 1], s_col[:, :])
            s_cur = s_next

        for g in range(G):
            prev = V_cur[g][0] if V_cur[g] is not None else None
            squash_group(s_cur, g, prev, is_final=True)

    nc.compile()
    return nc


_NC_CACHE = {}


def _get_nc(key):
    if key not in _NC_CACHE:
        _NC_CACHE[key] = _build(*key)
    return _NC_CACHE[key]


def _prep(u, weight, bias, c0, routings):
    u = np.ascontiguousarray(np.asarray(u, dtype=np.float32))
    weight = np.ascontiguousarray(np.asarray(weight, dtype=np.float32).reshape(weight.shape[-3:]))
    bias = np.ascontiguousarray(np.asarray(bias, dtype=np.float32).reshape(bias.shape[-2:]))
    c0 = np.ascontiguousarray(np.asarray(c0, dtype=np.float32).reshape(c0.shape[-2:]))
    routings = int(routings)
    B, NUM, IN_F = u.shape
    OUT_F = weight.shape[-1]
    uniform = bool(np.all(c0 == c0.flat[0]))
    c00 = float(c0.flat[0])
    assert B % N_CORES == 0, f"B={B} not divisible by {N_CORES}"
    B_core = B // N_CORES
    key = (B_core, NUM, IN_F, OUT_F, routings, c00 if uniform else 0.0, uniform)
    return u, weight, bias, c0, routings, B_core, key, uniform


def run_on_hw(u, weight, bias, c0, routings, trace=False):
    """Shard over cores, run SPMD, gather. Returns (out, exec_time_ns|None)."""
    u, weight, bias, c0, routings, B_core, key, uniform = _prep(
        u, weight, bias, c0, routings
    )
    nc = _get_nc(key)
    in_maps = []
    for c in range(N_CORES):
        m = {
            "u": u[c * B_core : (c + 1) * B_core],
            "w": weight,
            "bias": bias,
        }
        if not uniform:
            m["c0"] = c0
        in_maps.append(m)
    res = run_bass_kernel_spmd(nc, in_maps, core_ids=list(range(N_CORES)), trace=trace)
    out = np.concatenate([res.results[c]["out"] for c in range(N_CORES)], axis=0)
    return out, res.exec_time_ns


def kernel(**inputs):
    out, _ = run_on_hw(
        inputs["u"],
        inputs["weight"],
        inputs["bias"],
        inputs["c0"],
        inputs["routings"],
    )
    return out


# revision 22
# speedup vs baseline: 1.3605x; 1.3605x over previous
"""Trainium2 Bass kernel for CapsuleParall dynamic routing.

Math (per (b, n) pair, u_hat[i,o] = u[i] * W[n][i,o]):
    s_1[o] = sum_i u_hat[i,o] * c0[i,o]
    v_k    = squash(s_k + bias)           (squash over o)
    V_k    = v_1 + ... + v_k              (cumulative; b == u_hat * V)
    c_k    = softmax_o(u_hat[i,o] * V_k[o])
    s_{k+1}[o] = sum_i u_hat[i,o] * c_k[i,o]
    out    = squash(s_routings + bias)

On-chip strategy (layout: i on partitions, free = (chunk, o)):
    e[i,o] = exp(u_hat[i,o] * V[o])  unnormalized (values are small, safe)
    Z[i]   = sum_o e[i,o]            (per-chunk tensor_scalar accum on DVE)
    s[o]   = sum_i (W[i,o]*e[i,o]) * (u[i]/Z[i])
The PE matmul (lhsT = W.e chunk, rhs = (u/Z) column) applies both the u
factor and the softmax normalization during the i-contraction.  Hot-path
tensors are bf16 (DVE 2x/4x modes); accumulations are fp32.

Sharding: data-parallel over batch B across 8 cores (4 batches/core).
"""

import sys

sys.path.insert(0, "/opt/trn_rl_repo")

from contextlib import ExitStack

import numpy as np
import ml_dtypes

import concourse.bass as bass
import concourse.bacc as bacc
import concourse.mybir as mybir
import concourse.tile as tile
from concourse import masks
from concourse.bass_utils import run_bass_kernel_spmd

F32 = mybir.dt.float32
BF16 = mybir.dt.bfloat16
EPS = 1e-5
N_CORES = 8

# engine-split knobs
UHAT_DVE_CHUNKS = 0   # u_hat chunks with index < this go to DVE, rest Pool
F_DVE_MOD = 4         # pairs with (p % F_DVE_MOD) == 0 run f-mult on DVE
WAVE = 4              # pairs per software-pipeline wave


def _build(B_core, NUM, IN_F, OUT_F, routings, c00, uniform_c0):
    """Build the per-core Bass module."""
    P = 128
    assert IN_F % P == 0
    T = IN_F // P                      # 9 i-chunks
    PAIRS = B_core * NUM               # 64 (b, n) pairs per core
    # squash groups must start at partition 0/32/64/96 (HW AP restriction)
    GP = 32 if (PAIRS % 32 == 0 and PAIRS > 32) else PAIRS
    G = PAIRS // GP
    mult = mybir.AluOpType.mult
    add = mybir.AluOpType.add

    nc = bacc.Bacc("TRN2", target_bir_lowering=False, debug=False)

    u_dram = nc.dram_tensor("u", [B_core, NUM, IN_F], F32, kind="ExternalInput")
    w_dram = nc.dram_tensor("wbf", [NUM, IN_F, OUT_F], BF16, kind="ExternalInput")
    b_dram = nc.dram_tensor("bias", [NUM, OUT_F], F32, kind="ExternalInput")
    if not uniform_c0:
        c0_dram = nc.dram_tensor("c0", [IN_F, OUT_F], F32, kind="ExternalInput")
    out_dram = nc.dram_tensor("out", [B_core, NUM, OUT_F], F32, kind="ExternalOutput")

    def bcast_mid(ap2d, n):
        # [P, F] -> [P, n, F] with the middle dim broadcast (stride 0)
        return bass.AP(ap2d.tensor, ap2d.offset, [ap2d.ap[0], [0, n], ap2d.ap[1]])

    with tile.TileContext(nc) as tc, ExitStack() as ctx:
        const = ctx.enter_context(tc.tile_pool(name="const", bufs=1))
        work = ctx.enter_context(tc.tile_pool(name="work", bufs=12))
        small = ctx.enter_context(tc.tile_pool(name="small", bufs=12))
        sall_pool = ctx.enter_context(tc.tile_pool(name="sall", bufs=2))
        sq_pool = ctx.enter_context(tc.tile_pool(name="sq", bufs=4))
        vflat_pool = ctx.enter_context(tc.tile_pool(name="vflat", bufs=2))
        psum_s = ctx.enter_context(
            tc.tile_pool(name="psum_s", bufs=2, space=bass.MemorySpace.PSUM)
        )
        psum_vb = ctx.enter_context(
            tc.tile_pool(name="psum_vb", bufs=2, space=bass.MemorySpace.PSUM)
        )
        psum_tr = ctx.enter_context(
            tc.tile_pool(name="psum_tr", bufs=2, space=bass.MemorySpace.PSUM)
        )

        # ---- resident tensors ----
        W_sb = const.tile([P, NUM, T, OUT_F], BF16)      # W[n][i,o], i = t*128+p
        u_nat = const.tile([PAIRS, IN_F], F32)           # natural row layout
        u_sb = const.tile([P, T, PAIRS], F32)            # u columns (i on partitions)
        u_bf = const.tile([P, T, PAIRS], BF16)
        uc_bf = const.tile([P, T, PAIRS], BF16)          # u * c00 (uniform-c0 path)
        bias_all = const.tile([PAIRS, OUT_F], F32)
        ident = const.tile([P, P], F32)
        ones_row = const.tile([1, P], F32)
        if not uniform_c0:
            c0_sb = const.tile([P, T, OUT_F], BF16)

        # ---- loads ----
        w_ap = w_dram.ap()
        # src AP dims: [p(128), n, t, o] in elements of w_dram [NUM, IN_F, OUT_F]
        w_src = bass.AP(
            w_ap.tensor,
            w_ap.offset,
            [[OUT_F, P], [IN_F * OUT_F, NUM], [P * OUT_F, T], [1, OUT_F]],
        )
        nc.sync.dma_start(W_sb[:, :, :, :], w_src)
        nc.sync.dma_start(u_nat[:, :], u_dram.ap().rearrange("b n i -> (b n) i"))
        for b in range(B_core):
            nc.sync.dma_start(bias_all[b * NUM : (b + 1) * NUM, :], b_dram.ap())
        if not uniform_c0:
            c_ap = c0_dram.ap()
            c_src = bass.AP(
                c_ap.tensor, c_ap.offset, [[OUT_F, P], [P * OUT_F, T], [1, OUT_F]]
            )
            c0f = const.tile([P, T, OUT_F], F32)
            nc.sync.dma_start(c0f[:, :, :], c_src)
            nc.vector.tensor_copy(c0_sb[:, :, :], c0f[:, :, :])
        masks.make_identity(nc, ident[:, :])
        nc.vector.memset(ones_row[:, :], 1.0)

        # u_nat [PAIRS, IN_F] -> u_sb [P, T, PAIRS] via PE transposes per chunk
        for t in range(T):
            tr = psum_tr.tile([P, PAIRS], F32, tag="tr")
            nc.tensor.transpose(
                tr[:, :], u_nat[:, t * P : (t + 1) * P], ident[:PAIRS, :PAIRS]
            )
            nc.vector.tensor_copy(u_sb[:, t, :], tr[:, :])
        nc.vector.tensor_copy(u_bf[:, :, :], u_sb[:, :, :])
        nc.vector.tensor_scalar_mul(uc_bf[:, :, :], u_bf[:, :, :], float(c00))

        # ---- phase 1: s_1 for every pair ----
        s_allT = sall_pool.tile([P, PAIRS], F32, tag="sall")
        for p in range(PAIRS):
            n = p % NUM
            s_col = psum_s.tile([OUT_F, 1], F32, tag="scol")
            for t in range(T):
                if uniform_c0:
                    nc.tensor.matmul(
                        s_col[:, :],
                        W_sb[:, n, t, :],
                        uc_bf[:, t, p : p + 1],
                        start=(t == 0),
                        stop=(t == T - 1),
                    )
                else:
                    wc = work.tile([P, OUT_F], BF16, tag="wc0")
                    nc.vector.tensor_tensor(
                        wc[:, :], W_sb[:, n, t, :], c0_sb[:, t, :], op=mult
                    )
                    nc.tensor.matmul(
                        s_col[:, :],
                        wc[:, :],
                        u_bf[:, t, p : p + 1],
                        start=(t == 0),
                        stop=(t == T - 1),
                    )
            nc.vector.tensor_copy(s_allT[:, p : p + 1], s_col[:, :])

        # ---- squash (batched over a group of GP pairs) ----
        def squash_group(s_tile, g, V_prev, is_final):
            r0, r1 = g * GP, (g + 1) * GP
            tr = psum_tr.tile([GP, OUT_F], F32, tag="tr")
            nc.tensor.transpose(tr[:, :], s_tile[:, r0:r1], ident[:, :])
            sb = sq_pool.tile([GP, OUT_F], F32, tag="sb")
            nc.vector.tensor_tensor(sb[:, :], tr[:, :], bias_all[r0:r1, :], op=add)
            sqs = sq_pool.tile([GP, OUT_F], F32, tag="sqs")
            n2 = sq_pool.tile([GP, 1], F32, tag="n2")
            nc.vector.tensor_tensor(sqs[:, :], sb[:, :], sb[:, :], op=mult)
            nc.vector.tensor_reduce(
                n2[:, :], sqs[:, :], axis=mybir.AxisListType.X, op=add
            )
            rt = sq_pool.tile([GP, 1], F32, tag="rt")
            nc.scalar.activation(rt[:, :], n2[:, :], mybir.ActivationFunctionType.Sqrt)
            d1 = sq_pool.tile([GP, 1], F32, tag="d1")
            nc.vector.tensor_scalar_add(d1[:, :], n2[:, :], 1.0)
            d2 = sq_pool.tile([GP, 1], F32, tag="d2")
            nc.vector.tensor_scalar_add(d2[:, :], rt[:, :], EPS)
            den = sq_pool.tile([GP, 1], F32, tag="den")
            nc.vector.tensor_tensor(den[:, :], d1[:, :], d2[:, :], op=mult)
            rden = sq_pool.tile([GP, 1], F32, tag="rden")
            nc.vector.reciprocal(rden[:, :], den[:, :])
            coef = sq_pool.tile([GP, 1], F32, tag="coef")
            nc.vector.tensor_tensor(coef[:, :], n2[:, :], rden[:, :], op=mult)
            v = sq_pool.tile([GP, OUT_F], F32, tag="v")
            nc.vector.tensor_scalar_mul(v[:, :], sb[:, :], coef[:, 0:1])
            if is_final:
                out_rows = out_dram.ap().rearrange("b n o -> (b n) o")
                nc.sync.dma_start(out_rows[r0:r1, :], v[:, :])
                return None
            if V_prev is None:
                V_new = v
            else:
                V_new = sq_pool.tile([GP, OUT_F], F32, tag="V")
                nc.vector.tensor_tensor(V_new[:, :], V_prev[:, :], v[:, :], op=add)
            return V_new

        # ---- routing iterations (software-pipelined waves of WAVE pairs) ----
        V_cur = [None] * G
        s_cur = s_allT
        for k in range(2, routings + 1):
            s_next = sall_pool.tile([P, PAIRS], F32, tag="sall")
            for g in range(G):
                V_cur[g] = squash_group(s_cur, g, V_cur[g], is_final=False)
            nwaves = PAIRS // WAVE
            stage_fns = []

            def make_wave(w0):
                g = w0 // GP
                gl0 = w0 % GP
                NW = WAVE * OUT_F
                state = {}

                def s0():
                    V_flat = vflat_pool.tile([1, NW], F32, tag="vflat")
                    nc.sync.dma_start(V_flat[0:1, :], V_cur[g][gl0 : gl0 + WAVE, :])
                    vb_ps = psum_vb.tile([P, NW], F32, tag="vb")
                    for j in range(0, NW, 512):
                        jn = min(512, NW - j)
                        nc.tensor.matmul(
                            vb_ps[:, j : j + jn],
                            ones_row[:, :],
                            V_flat[0:1, j : j + jn],
                            start=True,
                            stop=True,
                        )
                    vb16 = small.tile([P, WAVE, OUT_F], BF16, tag="vb16")
                    nc.scalar.copy(vb16[:, :, :], vb_ps[:, :])
                    uh = []
                    for pl in range(WAVE):
                        p = w0 + pl
                        n = p % NUM
                        u_hat = work.tile([P, T, OUT_F], BF16, tag="uhat")
                        for t in range(T):
                            eng = nc.vector if t < UHAT_DVE_CHUNKS else nc.gpsimd
                            eng.tensor_scalar_mul(
                                u_hat[:, t, :], W_sb[:, n, t, :], u_sb[:, t, p : p + 1]
                            )
                        uh.append(u_hat)
                    state["vb16"] = vb16
                    state["uh"] = uh

                def s1():
                    tts = []
                    for pl in range(WAVE):
                        tt_ = work.tile([P, T, OUT_F], BF16, tag="tt")
                        nc.vector.tensor_tensor(
                            tt_[:, :, :],
                            state["uh"][pl][:, :, :],
                            bcast_mid(state["vb16"][:, pl, :], T),
                            op=mult,
                        )
                        tts.append(tt_)
                    state["tts"] = tts

                def s2():
                    ets = []
                    for pl in range(WAVE):
                        et = work.tile([P, T, OUT_F], BF16, tag="e")
                        nc.scalar.activation(
                            et[:, :, :],
                            state["tts"][pl][:, :, :],
                            mybir.ActivationFunctionType.Exp,
                        )
                        ets.append(et)
                    state["ets"] = ets

                def s3():
                    uws = []
                    for pl in range(WAVE):
                        p = w0 + pl
                        Z = small.tile([P, T], F32, tag="Z")
                        et = state["ets"][pl]
                        for t in range(T):
                            nc.vector.tensor_scalar(
                                et[:, t, :],
                                et[:, t, :],
                                1.0,
                                None,
                                mult,
                                op1=add,
                                accum_out=Z[:, t : t + 1],
                            )
                        wr = small.tile([P, T], F32, tag="wr")
                        nc.vector.reciprocal(wr[:, :], Z[:, :])
                        uw = small.tile([P, T], BF16, tag="uw")
                        nc.vector.tensor_tensor(
                            uw[:, :], wr[:, :], u_sb[:, :, p], op=mult
                        )
                        uws.append(uw)
                    state["uws"] = uws

                def s4():
                    s_ps = psum_s.tile([OUT_F, WAVE], F32, tag="scol")
                    for pl in range(WAVE):
                        p = w0 + pl
                        n = p % NUM
                        ft = work.tile([P, T, OUT_F], BF16, tag="tt")
                        eng = nc.vector if (p % F_DVE_MOD) == 0 else nc.gpsimd
                        eng.tensor_tensor(
                            ft[:, :, :],
                            state["ets"][pl][:, :, :],
                            W_sb[:, n, :, :],
                            op=mult,
                        )
                        for t in range(T):
                            nc.tensor.matmul(
                                s_ps[:, pl : pl + 1],
                                ft[:, t, :],
                                state["uws"][pl][:, t : t + 1],
                                start=(t == 0),
                                stop=(t == T - 1),
                            )
                    nc.vector.tensor_copy(s_next[:, w0 : w0 + WAVE], s_ps[:, :])

                return [s0, s1, s2, s3, s4]

            waves = [make_wave(w * WAVE) for w in range(nwaves)]
            NSTAGE = 5
            for step in range(nwaves + NSTAGE - 1):
                for st in range(NSTAGE - 1, -1, -1):
                    w = step - st
                    if 0 <= w < nwaves:
                        waves[w][st]()
            s_cur = s_next

        for g in range(G):
            squash_group(s_cur, g, V_cur[g], is_final=True)

    nc.compile()
    return nc


_NC_CACHE = {}


def _get_nc(key):
    if key not in _NC_CACHE:
        _NC_CACHE[key] = _build(*key)
    return _NC_CACHE[key]


def _prep(u, weight, bias, c0, routings):
    u = np.ascontiguousarray(np.asarray(u, dtype=np.float32))
    weight = np.ascontiguousarray(
        np.asarray(weight, dtype=np.float32).reshape(weight.shape[-3:])
    )
    bias = np.ascontiguousarray(np.asarray(bias, dtype=np.float32).reshape(bias.shape[-2:]))
    c0 = np.ascontiguousarray(np.asarray(c0, dtype=np.float32).reshape(c0.shape[-2:]))
    routings = int(routings)
    B, NUM, IN_F = u.shape
    OUT_F = weight.shape[-1]
    uniform = bool(np.all(c0 == c0.flat[0]))
    c00 = float(c0.flat[0])
    assert B % N_CORES == 0, f"B={B} not divisible by {N_CORES}"
    B_core = B // N_CORES
    key = (B_core, NUM, IN_F, OUT_F, routings, c00 if uniform else 0.0, uniform)
    return u, weight, bias, c0, routings, B_core, key, uniform


def run_on_hw(u, weight, bias, c0, routings, trace=False):
    """Shard over cores, run SPMD, gather. Returns (out, exec_time_ns|None)."""
    u, weight, bias, c0, routings, B_core, key, uniform = _prep(
        u, weight, bias, c0, routings
    )
    nc = _get_nc(key)
    wbf = weight.astype(ml_dtypes.bfloat16)
    in_maps = []
    for c in range(N_CORES):
        m = {
            "u": u[c * B_core : (c + 1) * B_core],
            "wbf": wbf,
            "bias": bias,
        }
        if not uniform:
            m["c0"] = c0
        in_maps.append(m)
    res = run_bass_kernel_spmd(nc, in_maps, core_ids=list(range(N_CORES)), trace=trace)
    out = np.concatenate([res.results[c]["out"] for c in range(N_CORES)], axis=0)
    return out, res.exec_time_ns


def kernel(**inputs):
    out, _ = run_on_hw(
        inputs["u"],
        inputs["weight"],
        inputs["bias"],
        inputs["c0"],
        inputs["routings"],
    )
    return out


# revision 25
# speedup vs baseline: 2370.7596x; 1742.5898x over previous
"""Trainium2 Bass kernel for CapsuleParall dynamic routing.

Math (per (b, n) pair, u_hat[i,o] = u[i] * W[n][i,o]):
    s_1[o] = sum_i u_hat[i,o] * c0[i,o]
    v_k    = squash(s_k + bias)           (squash over o)
    V_k    = v_1 + ... + v_k              (cumulative; b == u_hat * V)
    c_k    = softmax_o(u_hat[i,o] * V_k[o])
    s_{k+1}[o] = sum_i u_hat[i,o] * c_k[i,o]
    out    = squash(s_routings + bias)

On-chip strategy (layout: i on partitions, free = (chunk, o)):
    e[i,o] = exp(u_hat[i,o] * V[o])  unnormalized (values are small, safe)
    Z[i]   = sum_o e[i,o]            (per-chunk tensor_scalar accum on DVE)
    s[o]   = sum_i (W[i,o]*e[i,o]) * (u[i]/Z[i])
The PE matmul (lhsT = W.e chunk, rhs = (u/Z) column) applies both the u
factor and the softmax normalization during the i-contraction.  Hot-path
tensors are bf16 (DVE 2x/4x modes); accumulations are fp32.

Sharding: data-parallel over batch B across 8 cores (4 batches/core).
"""

import sys

sys.path.insert(0, "/opt/trn_rl_repo")

from contextlib import ExitStack

import numpy as np
import ml_dtypes

import concourse.bass as bass
import concourse.bacc as bacc
import concourse.mybir as mybir
import concourse.tile as tile
from concourse import masks
from concourse.bass_utils import run_bass_kernel_spmd

F32 = mybir.dt.float32
BF16 = mybir.dt.bfloat16
EPS = 1e-5
N_CORES = 8

# engine-split knobs
UHAT_DVE_CHUNKS = 0   # u_hat chunks with index < this go to DVE, rest Pool
F_DVE_MOD = 4         # pairs with (p % F_DVE_MOD) == 0 run f-mult on DVE
WAVE = 4              # pairs per software-pipeline wave


def _build(B_core, NUM, IN_F, OUT_F, routings, c00, uniform_c0):
    """Build the per-core Bass module."""
    P = 128
    assert IN_F % P == 0
    T = IN_F // P                      # 9 i-chunks
    PAIRS = B_core * NUM               # 64 (b, n) pairs per core
    # squash groups must start at partition 0/32/64/96 (HW AP restriction)
    GP = 32 if (PAIRS % 32 == 0 and PAIRS > 32) else PAIRS
    G = PAIRS // GP
    mult = mybir.AluOpType.mult
    add = mybir.AluOpType.add

    nc = bacc.Bacc("TRN2", target_bir_lowering=False, debug=False)

    u_dram = nc.dram_tensor("u", [B_core, NUM, IN_F], F32, kind="ExternalInput")
    w_dram = nc.dram_tensor("wbf", [NUM, IN_F, OUT_F], BF16, kind="ExternalInput")
    b_dram = nc.dram_tensor("bias", [NUM, OUT_F], F32, kind="ExternalInput")
    if not uniform_c0:
        c0_dram = nc.dram_tensor("c0", [IN_F, OUT_F], F32, kind="ExternalInput")
    out_dram = nc.dram_tensor("out", [B_core, NUM, OUT_F], F32, kind="ExternalOutput")

    def bcast_mid(ap2d, n):
        # [P, F] -> [P, n, F] with the middle dim broadcast (stride 0)
        return bass.AP(ap2d.tensor, ap2d.offset, [ap2d.ap[0], [0, n], ap2d.ap[1]])

    with tile.TileContext(nc) as tc, ExitStack() as ctx:
        const = ctx.enter_context(tc.tile_pool(name="const", bufs=1))
        work = ctx.enter_context(tc.tile_pool(name="work", bufs=12))
        small = ctx.enter_context(tc.tile_pool(name="small", bufs=12))
        sall_pool = ctx.enter_context(tc.tile_pool(name="sall", bufs=2))
        sq_pool = ctx.enter_context(tc.tile_pool(name="sq", bufs=4))
        vflat_pool = ctx.enter_context(tc.tile_pool(name="vflat", bufs=2))
        psum_s = ctx.enter_context(
            tc.tile_pool(name="psum_s", bufs=2, space=bass.MemorySpace.PSUM)
        )
        psum_vb = ctx.enter_context(
            tc.tile_pool(name="psum_vb", bufs=2, space=bass.MemorySpace.PSUM)
        )
        psum_tr = ctx.enter_context(
            tc.tile_pool(name="psum_tr", bufs=2, space=bass.MemorySpace.PSUM)
        )

        # ---- resident tensors ----
        W_sb = const.tile([P, NUM, T, OUT_F], BF16)      # W[n][i,o], i = t*128+p
        u_nat = const.tile([PAIRS, IN_F], F32)           # natural row layout
        u_sb = const.tile([P, T, PAIRS], F32)            # u columns (i on partitions)
        u_bf = const.tile([P, T, PAIRS], BF16)
        uc_bf = const.tile([P, T, PAIRS], BF16)          # u * c00 (uniform-c0 path)
        bias_all = const.tile([PAIRS, OUT_F], F32)
        ident = const.tile([P, P], F32)
        ones_row = const.tile([1, P], F32)
        if not uniform_c0:
            c0_sb = const.tile([P, T, OUT_F], BF16)

        # ---- loads ----
        w_ap = w_dram.ap()
        # src AP dims: [p(128), n, t, o] in elements of w_dram [NUM, IN_F, OUT_F]
        w_src = bass.AP(
            w_ap.tensor,
            w_ap.offset,
            [[OUT_F, P], [IN_F * OUT_F, NUM], [P * OUT_F, T], [1, OUT_F]],
        )
        nc.sync.dma_start(W_sb[:, :, :, :], w_src)
        nc.sync.dma_start(u_nat[:, :], u_dram.ap().rearrange("b n i -> (b n) i"))
        for b in range(B_core):
            nc.sync.dma_start(bias_all[b * NUM : (b + 1) * NUM, :], b_dram.ap())
        if not uniform_c0:
            c_ap = c0_dram.ap()
            c_src = bass.AP(
                c_ap.tensor, c_ap.offset, [[OUT_F, P], [P * OUT_F, T], [1, OUT_F]]
            )
            c0f = const.tile([P, T, OUT_F], F32)
            nc.sync.dma_start(c0f[:, :, :], c_src)
            nc.vector.tensor_copy(c0_sb[:, :, :], c0f[:, :, :])
        masks.make_identity(nc, ident[:, :])
        nc.vector.memset(ones_row[:, :], 1.0)

        # u_nat [PAIRS, IN_F] -> u_sb [P, T, PAIRS] via PE transposes per chunk
        for t in range(T):
            tr = psum_tr.tile([P, PAIRS], F32, tag="tr")
            nc.tensor.transpose(
                tr[:, :], u_nat[:, t * P : (t + 1) * P], ident[:PAIRS, :PAIRS]
            )
            nc.vector.tensor_copy(u_sb[:, t, :], tr[:, :])
        nc.vector.tensor_copy(u_bf[:, :, :], u_sb[:, :, :])
        nc.vector.tensor_scalar_mul(uc_bf[:, :, :], u_bf[:, :, :], float(c00))

        # ---- phase 1: s_1 for every pair ----
        s_allT = sall_pool.tile([P, PAIRS], F32, tag="sall")
        for p in range(PAIRS):
            n = p % NUM
            s_col = psum_s.tile([OUT_F, 1], F32, tag="scol")
            for t in range(T):
                if uniform_c0:
                    nc.tensor.matmul(
                        s_col[:, :],
                        W_sb[:, n, t, :],
                        uc_bf[:, t, p : p + 1],
                        start=(t == 0),
                        stop=(t == T - 1),
                    )
                else:
                    wc = work.tile([P, OUT_F], BF16, tag="wc0")
                    nc.vector.tensor_tensor(
                        wc[:, :], W_sb[:, n, t, :], c0_sb[:, t, :], op=mult
                    )
                    nc.tensor.matmul(
                        s_col[:, :],
                        wc[:, :],
                        u_bf[:, t, p : p + 1],
                        start=(t == 0),
                        stop=(t == T - 1),
                    )
            nc.vector.tensor_copy(s_allT[:, p : p + 1], s_col[:, :])

        # ---- squash (batched over a group of GP pairs) ----
        def squash_group(s_tile, g, V_prev, is_final):
            r0, r1 = g * GP, (g + 1) * GP
            tr = psum_tr.tile([GP, OUT_F], F32, tag="tr")
            nc.tensor.transpose(tr[:, :], s_tile[:, r0:r1], ident[:, :])
            sb = sq_pool.tile([GP, OUT_F], F32, tag="sb")
            nc.vector.tensor_tensor(sb[:, :], tr[:, :], bias_all[r0:r1, :], op=add)
            sqs = sq_pool.tile([GP, OUT_F], F32, tag="sqs")
            n2 = sq_pool.tile([GP, 1], F32, tag="n2")
            nc.vector.tensor_tensor(sqs[:, :], sb[:, :], sb[:, :], op=mult)
            nc.vector.tensor_reduce(
                n2[:, :], sqs[:, :], axis=mybir.AxisListType.X, op=add
            )
            rt = sq_pool.tile([GP, 1], F32, tag="rt")
            nc.scalar.activation(rt[:, :], n2[:, :], mybir.ActivationFunctionType.Sqrt)
            d1 = sq_pool.tile([GP, 1], F32, tag="d1")
            nc.vector.tensor_scalar_add(d1[:, :], n2[:, :], 1.0)
            d2 = sq_pool.tile([GP, 1], F32, tag="d2")
            nc.vector.tensor_scalar_add(d2[:, :], rt[:, :], EPS)
            den = sq_pool.tile([GP, 1], F32, tag="den")
            nc.vector.tensor_tensor(den[:, :], d1[:, :], d2[:, :], op=mult)
            rden = sq_pool.tile([GP, 1], F32, tag="rden")
            nc.vector.reciprocal(rden[:, :], den[:, :])
            coef = sq_pool.tile([GP, 1], F32, tag="coef")
            nc.vector.tensor_tensor(coef[:, :], n2[:, :], rden[:, :], op=mult)
            v = sq_pool.tile([GP, OUT_F], F32, tag="v")
            nc.vector.tensor_scalar_mul(v[:, :], sb[:, :], coef[:, 0:1])
            if is_final:
                out_rows = out_dram.ap().rearrange("b n o -> (b n) o")
                nc.sync.dma_start(out_rows[r0:r1, :], v[:, :])
                return None
            if V_prev is None:
                V_new = v
            else:
                V_new = sq_pool.tile([GP, OUT_F], F32, tag="V")
                nc.vector.tensor_tensor(V_new[:, :], V_prev[:, :], v[:, :], op=add)
            return V_new

        # ---- routing iterations (software-pipelined waves of WAVE pairs) ----
        V_cur = [None] * G
        s_cur = s_allT
        for k in range(2, routings + 1):
            s_next = sall_pool.tile([P, PAIRS], F32, tag="sall")
            for g in range(G):
                V_cur[g] = squash_group(s_cur, g, V_cur[g], is_final=False)
            nwaves = PAIRS // WAVE
            stage_fns = []

            def make_wave(w0):
                g = w0 // GP
                gl0 = w0 % GP
                NW = WAVE * OUT_F
                state = {}

                def s0():
                    V_flat = vflat_pool.tile([1, NW], F32, tag="vflat")
                    nc.sync.dma_start(V_flat[0:1, :], V_cur[g][gl0 : gl0 + WAVE, :])
                    vb_ps = psum_vb.tile([P, NW], F32, tag="vb")
                    for j in range(0, NW, 512):
                        jn = min(512, NW - j)
                        nc.tensor.matmul(
                            vb_ps[:, j : j + jn],
                            ones_row[:, :],
                            V_flat[0:1, j : j + jn],
                            start=True,
                            stop=True,
                        )
                    vb16 = small.tile([P, WAVE, OUT_F], BF16, tag="vb16")
                    nc.scalar.copy(vb16[:, :, :], vb_ps[:, :])
                    uh = []
                    for pl in range(WAVE):
                        p = w0 + pl
                        n = p % NUM
                        u_hat = work.tile([P, T, OUT_F], BF16, tag="uhat")
                        for t in range(T):
                            eng = nc.vector if t < UHAT_DVE_CHUNKS else nc.gpsimd
                            eng.tensor_scalar_mul(
                                u_hat[:, t, :], W_sb[:, n, t, :], u_sb[:, t, p : p + 1]
                            )
                        uh.append(u_hat)
                    state["vb16"] = vb16
                    state["uh"] = uh

                def s1():
                    tts = []
                    for pl in range(WAVE):
                        tt_ = work.tile([P, T, OUT_F], BF16, tag="tt")
                        nc.vector.tensor_tensor(
                            tt_[:, :, :],
                            state["uh"][pl][:, :, :],
                            bcast_mid(state["vb16"][:, pl, :], T),
                            op=mult,
                        )
                        tts.append(tt_)
                    state["tts"] = tts

                def s2():
                    ets = []
                    for pl in range(WAVE):
                        et = work.tile([P, T, OUT_F], BF16, tag="e")
                        nc.scalar.activation(
                            et[:, :, :],
                            state["tts"][pl][:, :, :],
                            mybir.ActivationFunctionType.Exp,
                        )
                        ets.append(et)
                    state["ets"] = ets

                def s3():
                    uws = []
                    for pl in range(WAVE):
                        p = w0 + pl
                        Z = small.tile([P, T], F32, tag="Z")
                        et = state["ets"][pl]
                        for t in range(T):
                            nc.vector.tensor_scalar(
                                et[:, t, :],
                                et[:, t, :],
                                1.0,
                                None,
                                mult,
                                op1=add,
                                accum_out=Z[:, t : t + 1],
                            )
                        wr = small.tile([P, T], F32, tag="wr")
                        nc.vector.reciprocal(wr[:, :], Z[:, :])
                        uw = small.tile([P, T], BF16, tag="uw")
                        nc.vector.tensor_tensor(
                            uw[:, :], wr[:, :], u_sb[:, :, p], op=mult
                        )
                        uws.append(uw)
                    state["uws"] = uws

                def s4():
                    s_ps = psum_s.tile([OUT_F, WAVE], F32, tag="scol")
                    for pl in range(WAVE):
                        p = w0 + pl
                        n = p % NUM
                        ft = work.tile([P, T, OUT_F], BF16, tag="tt")
                        eng = nc.vector if (p % F_DVE_MOD) == 0 else nc.gpsimd
                        eng.tensor_tensor(
                            ft[:, :, :],
                            state["ets"][pl][:, :, :],
                            W_sb[:, n, :, :],
                            op=mult,
                        )
                        for t in range(T):
                            nc.tensor.matmul(
                                s_ps[:, pl : pl + 1],
                                ft[:, t, :],
                                state["uws"][pl][:, t : t + 1],
                                start=(t == 0),
                                stop=(t == T - 1),
                            )
                    nc.vector.tensor_copy(s_next[:, w0 : w0 + WAVE], s_ps[:, :])

                return [s0, s1, s2, s3, s4]

            waves = [make_wave(w * WAVE) for w in range(nwaves)]
            NSTAGE = 5
            for step in range(nwaves + NSTAGE - 1):
                for st in range(NSTAGE - 1, -1, -1):
                    w = step - st
                    if 0 <= w < nwaves:
                        waves[w][st]()
            s_cur = s_next

        for g in range(G):
            squash_group(s_cur, g, V_cur[g], is_final=True)

    nc.compile()
    return nc


_NC_CACHE = {}


def _get_nc(key):
    if key not in _NC_CACHE:
        _NC_CACHE[key] = _build(*key)
    return _NC_CACHE[key]


def _prep(u, weight, bias, c0, routings):
    u = np.ascontiguousarray(np.asarray(u, dtype=np.float32))
    weight = np.ascontiguousarray(
        np.asarray(weight, dtype=np.float32).reshape(weight.shape[-3:])
    )
    bias = np.ascontiguousarray(np.asarray(bias, dtype=np.float32).reshape(bias.shape[-2:]))
    c0 = np.ascontiguousarray(np.asarray(c0, dtype=np.float32).reshape(c0.shape[-2:]))
    routings = int(routings)
    B, NUM, IN_F = u.shape
    OUT_F = weight.shape[-1]
    uniform = bool(np.all(c0 == c0.flat[0]))
    c00 = float(c0.flat[0])
    assert B % N_CORES == 0, f"B={B} not divisible by {N_CORES}"
    B_core = B // N_CORES
    key = (B_core, NUM, IN_F, OUT_F, routings, c00 if uniform else 0.0, uniform)
    return u, weight, bias, c0, routings, B_core, key, uniform


def run_on_hw(u, weight, bias, c0, routings, trace=False):
    """Shard over cores, run SPMD, gather. Returns (out, exec_time_ns|None)."""
    u, weight, bias, c0, routings, B_core, key, uniform = _prep(
        u, weight, bias, c0, routings
    )
    nc = _get_nc(key)
    wbf = weight.astype(ml_dtypes.bfloat16)
    in_maps = []
    for c in range(N_CORES):
        m = {
            "u": u[c * B_core : (c + 1) * B_core],
            "wbf": wbf,
            "bias": bias,
        }
        if not uniform:
            m["c0"] = c0
        in_maps.append(m)
    res = run_bass_kernel_spmd(nc, in_maps, core_ids=list(range(N_CORES)), trace=trace)
    out = np.concatenate([res.results[c]["out"] for c in range(N_CORES)], axis=0)
    return out, res.exec_time_ns


_RUNNER_CACHE = {}


def _get_runner(key):
    """Cached jitted multi-core executable (avoids per-call re-jit)."""
    if key in _RUNNER_CACHE:
        return _RUNNER_CACHE[key]
    import jax
    from jax.sharding import Mesh, PartitionSpec
    from jax.experimental.shard_map import shard_map
    from concourse import bass2jax, mybir as mb

    nc = _get_nc(key)
    bass2jax.install_neuronx_cc_hook()
    part_name = nc.partition_id_tensor.name if nc.partition_id_tensor else None
    in_names, out_names, out_avals, zero_outs = [], [], [], []
    for alloc in nc.m.functions[0].allocations:
        if not isinstance(alloc, mb.MemoryLocationSet):
            continue
        name = alloc.memorylocations[0].name
        if alloc.kind == "ExternalInput":
            if name != part_name:
                in_names.append(name)
        elif alloc.kind == "ExternalOutput":
            out_names.append(name)
            shape = tuple(alloc.tensor_shape)
            dtype = mb.dt.np(alloc.dtype)
            out_avals.append(jax.core.ShapedArray(shape, dtype))
            zero_outs.append(np.zeros(shape, dtype))
    n_params = len(in_names)
    all_names = in_names + out_names
    if part_name is not None:
        all_names = all_names + [part_name]
    donate = tuple(range(n_params, n_params + len(out_names)))

    def _body(*args):
        operands = list(args)
        if part_name is not None:
            operands.append(bass2jax.partition_id_tensor())
        outs = bass2jax._bass_exec_p.bind(
            *operands,
            out_avals=tuple(out_avals),
            in_names=tuple(all_names),
            out_names=tuple(out_names),
            lowering_input_output_aliases=(),
            sim_require_finite=True,
            sim_require_nnan=True,
            nc=nc,
        )
        return tuple(outs)

    devices = jax.devices()[:N_CORES]
    mesh = Mesh(np.asarray(devices), ("core",))
    specs = (PartitionSpec("core"),) * (n_params + len(out_names))
    fn = jax.jit(
        shard_map(
            _body,
            mesh=mesh,
            in_specs=specs,
            out_specs=(PartitionSpec("core"),) * len(out_names),
            check_rep=False,
        ),
        donate_argnums=donate,
        keep_unused=True,
    )
    runner = (fn, in_names, out_names, out_avals, zero_outs)
    _RUNNER_CACHE[key] = runner
    return runner


def run_cached(u, weight, bias, c0, routings):
    """Run via a cached jitted executable. Returns (out, per_call_fn)."""
    u, weight, bias, c0, routings, B_core, key, uniform = _prep(
        u, weight, bias, c0, routings
    )
    fn, in_names, out_names, out_avals, zero_outs = _get_runner(key)
    wbf = weight.astype(ml_dtypes.bfloat16)
    per_core = {
        "u": [u[c * B_core : (c + 1) * B_core] for c in range(N_CORES)],
        "wbf": [wbf] * N_CORES,
        "bias": [bias] * N_CORES,
        "c0": [c0] * N_CORES,
    }
    concat_in = [np.concatenate(per_core[nm], axis=0) for nm in in_names]

    def call():
        zeros = [
            np.zeros((N_CORES * z.shape[0], *z.shape[1:]), z.dtype)
            for z in zero_outs
        ]
        outs = fn(*concat_in, *zeros)
        return np.asarray(outs[0])

    full = call()
    i = out_names.index("out")
    B_total = N_CORES * B_core
    out = full.reshape(N_CORES, B_core, *out_avals[i].shape[1:]).reshape(
        B_total, *out_avals[i].shape[1:]
    )
    return out, call


def kernel(**inputs):
    out, _ = run_on_hw(
        inputs["u"],
        inputs["weight"],
        inputs["bias"],
        inputs["c0"],
        inputs["routings"],
    )
    return out


# revision 26
# speedup vs baseline: 2398.8573x; 1.0119x over previous
"""Trainium2 Bass kernel for CapsuleParall dynamic routing.

Math (per (b, n) pair, u_hat[i,o] = u[i] * W[n][i,o]):
    s_1[o] = sum_i u_hat[i,o] * c0[i,o]
    v_k    = squash(s_k + bias)           (squash over o)
    V_k    = v_1 + ... + v_k              (cumulative; b == u_hat * V)
    c_k    = softmax_o(u_hat[i,o] * V_k[o])
    s_{k+1}[o] = sum_i u_hat[i,o] * c_k[i,o]
    out    = squash(s_routings + bias)

On-chip strategy (layout: i on partitions, free = (chunk, o)):
    e[i,o] = exp(u_hat[i,o] * V[o])  unnormalized (values are small, safe)
    Z[i]   = sum_o e[i,o]            (per-chunk tensor_scalar accum on DVE)
    s[o]   = sum_i (W[i,o]*e[i,o]) * (u[i]/Z[i])
The PE matmul (lhsT = W.e chunk, rhs = (u/Z) column) applies both the u
factor and the softmax normalization during the i-contraction.  Hot-path
tensors are bf16 (DVE 2x/4x modes); accumulations are fp32.

Sharding: data-parallel over batch B across 8 cores (4 batches/core).
"""

import sys

sys.path.insert(0, "/opt/trn_rl_repo")

from contextlib import ExitStack

import numpy as np
import ml_dtypes

import concourse.bass as bass
import concourse.bacc as bacc
import concourse.mybir as mybir
import concourse.tile as tile
from concourse import masks
from concourse.bass_utils import run_bass_kernel_spmd

F32 = mybir.dt.float32
BF16 = mybir.dt.bfloat16
EPS = 1e-5
N_CORES = 8

# engine-split knobs
UHAT_DVE_CHUNKS = 0   # u_hat chunks with index < this go to DVE, rest Pool
F_DVE_MOD = 4         # pairs with (p % F_DVE_MOD) == 0 run f-mult on DVE
WAVE = 4              # pairs per software-pipeline wave


def _build(B_core, NUM, IN_F, OUT_F, routings, c00, uniform_c0):
    """Build the per-core Bass module."""
    P = 128
    assert IN_F % P == 0
    T = IN_F // P                      # 9 i-chunks
    PAIRS = B_core * NUM               # 64 (b, n) pairs per core
    # squash groups must start at partition 0/32/64/96 (HW AP restriction)
    GP = 32 if (PAIRS % 32 == 0 and PAIRS > 32) else PAIRS
    G = PAIRS // GP
    mult = mybir.AluOpType.mult
    add = mybir.AluOpType.add

    nc = bacc.Bacc("TRN2", target_bir_lowering=False, debug=False)

    u_dram = nc.dram_tensor("u", [B_core, NUM, IN_F], F32, kind="ExternalInput")
    w_dram = nc.dram_tensor("wbf", [NUM, IN_F, OUT_F], BF16, kind="ExternalInput")
    b_dram = nc.dram_tensor("bias", [NUM, OUT_F], F32, kind="ExternalInput")
    if not uniform_c0:
        c0_dram = nc.dram_tensor("c0", [IN_F, OUT_F], F32, kind="ExternalInput")
    out_dram = nc.dram_tensor("out", [B_core, NUM, OUT_F], F32, kind="ExternalOutput")

    def bcast_mid(ap2d, n):
        # [P, F] -> [P, n, F] with the middle dim broadcast (stride 0)
        return bass.AP(ap2d.tensor, ap2d.offset, [ap2d.ap[0], [0, n], ap2d.ap[1]])

    with tile.TileContext(nc) as tc, ExitStack() as ctx:
        const = ctx.enter_context(tc.tile_pool(name="const", bufs=1))
        work = ctx.enter_context(tc.tile_pool(name="work", bufs=12))
        small = ctx.enter_context(tc.tile_pool(name="small", bufs=12))
        sall_pool = ctx.enter_context(tc.tile_pool(name="sall", bufs=2))
        sq_pool = ctx.enter_context(tc.tile_pool(name="sq", bufs=4))
        vflat_pool = ctx.enter_context(tc.tile_pool(name="vflat", bufs=2))
        psum_s = ctx.enter_context(
            tc.tile_pool(name="psum_s", bufs=2, space=bass.MemorySpace.PSUM)
        )
        psum_vb = ctx.enter_context(
            tc.tile_pool(name="psum_vb", bufs=2, space=bass.MemorySpace.PSUM)
        )
        psum_tr = ctx.enter_context(
            tc.tile_pool(name="psum_tr", bufs=2, space=bass.MemorySpace.PSUM)
        )

        # ---- resident tensors ----
        W_sb = const.tile([P, NUM, T, OUT_F], BF16)      # W[n][i,o], i = t*128+p
        u_nat = const.tile([PAIRS, IN_F], F32)           # natural row layout
        u_sb = const.tile([P, T, PAIRS], F32)            # u columns (i on partitions)
        u_bf = const.tile([P, T, PAIRS], BF16)
        uc_bf = const.tile([P, T, PAIRS], BF16)          # u * c00 (uniform-c0 path)
        bias_all = const.tile([PAIRS, OUT_F], F32)
        ident = const.tile([P, P], F32)
        ones_row = const.tile([1, P], F32)
        if not uniform_c0:
            c0_sb = const.tile([P, T, OUT_F], BF16)

        # ---- loads ----
        w_ap = w_dram.ap()
        # src AP dims: [p(128), n, t, o] in elements of w_dram [NUM, IN_F, OUT_F]
        w_src = bass.AP(
            w_ap.tensor,
            w_ap.offset,
            [[OUT_F, P], [IN_F * OUT_F, NUM], [P * OUT_F, T], [1, OUT_F]],
        )
        nc.sync.dma_start(W_sb[:, :, :, :], w_src)
        nc.sync.dma_start(u_nat[:, :], u_dram.ap().rearrange("b n i -> (b n) i"))
        for b in range(B_core):
            nc.sync.dma_start(bias_all[b * NUM : (b + 1) * NUM, :], b_dram.ap())
        if not uniform_c0:
            c_ap = c0_dram.ap()
            c_src = bass.AP(
                c_ap.tensor, c_ap.offset, [[OUT_F, P], [P * OUT_F, T], [1, OUT_F]]
            )
            c0f = const.tile([P, T, OUT_F], F32)
            nc.sync.dma_start(c0f[:, :, :], c_src)
            nc.vector.tensor_copy(c0_sb[:, :, :], c0f[:, :, :])
        masks.make_identity(nc, ident[:, :])
        nc.vector.memset(ones_row[:, :], 1.0)

        # u_nat [PAIRS, IN_F] -> u_sb [P, T, PAIRS] via PE transposes per chunk
        for t in range(T):
            tr = psum_tr.tile([P, PAIRS], F32, tag="tr")
            nc.tensor.transpose(
                tr[:, :], u_nat[:, t * P : (t + 1) * P], ident[:PAIRS, :PAIRS]
            )
            nc.vector.tensor_copy(u_sb[:, t, :], tr[:, :])
        nc.vector.tensor_copy(u_bf[:, :, :], u_sb[:, :, :])
        nc.vector.tensor_scalar_mul(uc_bf[:, :, :], u_bf[:, :, :], float(c00))

        # ---- phase 1: s_1 for every pair ----
        s_allT = sall_pool.tile([P, PAIRS], F32, tag="sall")
        for p in range(PAIRS):
            n = p % NUM
            s_col = psum_s.tile([OUT_F, 1], F32, tag="scol")
            for t in range(T):
                if uniform_c0:
                    nc.tensor.matmul(
                        s_col[:, :],
                        W_sb[:, n, t, :],
                        uc_bf[:, t, p : p + 1],
                        start=(t == 0),
                        stop=(t == T - 1),
                    )
                else:
                    wc = work.tile([P, OUT_F], BF16, tag="wc0")
                    nc.vector.tensor_tensor(
                        wc[:, :], W_sb[:, n, t, :], c0_sb[:, t, :], op=mult
                    )
                    nc.tensor.matmul(
                        s_col[:, :],
                        wc[:, :],
                        u_bf[:, t, p : p + 1],
                        start=(t == 0),
                        stop=(t == T - 1),
                    )
            nc.vector.tensor_copy(s_allT[:, p : p + 1], s_col[:, :])

        # ---- squash (batched over a group of GP pairs) ----
        def squash_group(s_tile, g, V_prev, is_final):
            r0, r1 = g * GP, (g + 1) * GP
            tr = psum_tr.tile([GP, OUT_F], F32, tag="tr")
            nc.tensor.transpose(tr[:, :], s_tile[:, r0:r1], ident[:, :])
            sb = sq_pool.tile([GP, OUT_F], F32, tag="sb")
            nc.vector.tensor_tensor(sb[:, :], tr[:, :], bias_all[r0:r1, :], op=add)
            sqs = sq_pool.tile([GP, OUT_F], F32, tag="sqs")
            n2 = sq_pool.tile([GP, 1], F32, tag="n2")
            nc.vector.tensor_tensor(sqs[:, :], sb[:, :], sb[:, :], op=mult)
            nc.vector.tensor_reduce(
                n2[:, :], sqs[:, :], axis=mybir.AxisListType.X, op=add
            )
            rt = sq_pool.tile([GP, 1], F32, tag="rt")
            nc.scalar.activation(rt[:, :], n2[:, :], mybir.ActivationFunctionType.Sqrt)
            d1 = sq_pool.tile([GP, 1], F32, tag="d1")
            nc.vector.tensor_scalar_add(d1[:, :], n2[:, :], 1.0)
            d2 = sq_pool.tile([GP, 1], F32, tag="d2")
            nc.vector.tensor_scalar_add(d2[:, :], rt[:, :], EPS)
            den = sq_pool.tile([GP, 1], F32, tag="den")
            nc.vector.tensor_tensor(den[:, :], d1[:, :], d2[:, :], op=mult)
            rden = sq_pool.tile([GP, 1], F32, tag="rden")
            nc.vector.reciprocal(rden[:, :], den[:, :])
            coef = sq_pool.tile([GP, 1], F32, tag="coef")
            nc.vector.tensor_tensor(coef[:, :], n2[:, :], rden[:, :], op=mult)
            v = sq_pool.tile([GP, OUT_F], F32, tag="v")
            nc.vector.tensor_scalar_mul(v[:, :], sb[:, :], coef[:, 0:1])
            if is_final:
                out_rows = out_dram.ap().rearrange("b n o -> (b n) o")
                nc.sync.dma_start(out_rows[r0:r1, :], v[:, :])
                return None
            if V_prev is None:
                V_new = v
            else:
                V_new = sq_pool.tile([GP, OUT_F], F32, tag="V")
                nc.vector.tensor_tensor(V_new[:, :], V_prev[:, :], v[:, :], op=add)
            return V_new

        # ---- routing iterations (software-pipelined waves of WAVE pairs) ----
        V_cur = [None] * G
        s_cur = s_allT
        for k in range(2, routings + 1):
            s_next = sall_pool.tile([P, PAIRS], F32, tag="sall")
            for g in range(G):
                V_cur[g] = squash_group(s_cur, g, V_cur[g], is_final=False)
            nwaves = PAIRS // WAVE
            stage_fns = []

            def make_wave(w0):
                g = w0 // GP
                gl0 = w0 % GP
                NW = WAVE * OUT_F
                state = {}

                def s0():
                    V_flat = vflat_pool.tile([1, NW], F32, tag="vflat")
                    nc.sync.dma_start(V_flat[0:1, :], V_cur[g][gl0 : gl0 + WAVE, :])
                    vb_ps = psum_vb.tile([P, NW], F32, tag="vb")
                    for j in range(0, NW, 512):
                        jn = min(512, NW - j)
                        nc.tensor.matmul(
                            vb_ps[:, j : j + jn],
                            ones_row[:, :],
                            V_flat[0:1, j : j + jn],
                            start=True,
                            stop=True,
                        )
                    vb16 = small.tile([P, WAVE, OUT_F], BF16, tag="vb16")
                    nc.scalar.copy(vb16[:, :, :], vb_ps[:, :])
                    uh = []
                    for pl in range(WAVE):
                        p = w0 + pl
                        n = p % NUM
                        u_hat = work.tile([P, T, OUT_F], BF16, tag="uhat")
                        ucol = u_bf[:, :, p : p + 1]
                        ub = bass.AP(
                            ucol.tensor, ucol.offset,
                            [ucol.ap[0], ucol.ap[1], [0, OUT_F]],
                        )
                        nc.gpsimd.tensor_tensor(
                            u_hat[:, :, :], W_sb[:, n, :, :], ub, op=mult
                        )
                        uh.append(u_hat)
                    state["vb16"] = vb16
                    state["uh"] = uh

                def s1():
                    tts = []
                    for pl in range(WAVE):
                        tt_ = work.tile([P, T, OUT_F], BF16, tag="tt")
                        nc.vector.tensor_tensor(
                            tt_[:, :, :],
                            state["uh"][pl][:, :, :],
                            bcast_mid(state["vb16"][:, pl, :], T),
                            op=mult,
                        )
                        tts.append(tt_)
                    state["tts"] = tts

                def s2():
                    ets = []
                    for pl in range(WAVE):
                        et = work.tile([P, T, OUT_F], BF16, tag="e")
                        nc.scalar.activation(
                            et[:, :, :],
                            state["tts"][pl][:, :, :],
                            mybir.ActivationFunctionType.Exp,
                        )
                        ets.append(et)
                    state["ets"] = ets

                def s3():
                    uws = []
                    for pl in range(WAVE):
                        p = w0 + pl
                        Z = small.tile([P, T], F32, tag="Z")
                        et = state["ets"][pl]
                        for t in range(T):
                            nc.vector.tensor_scalar(
                                et[:, t, :],
                                et[:, t, :],
                                1.0,
                                None,
                                mult,
                                op1=add,
                                accum_out=Z[:, t : t + 1],
                            )
                        wr = small.tile([P, T], F32, tag="wr")
                        nc.vector.reciprocal(wr[:, :], Z[:, :])
                        uw = small.tile([P, T], BF16, tag="uw")
                        nc.vector.tensor_tensor(
                            uw[:, :], wr[:, :], u_sb[:, :, p], op=mult
                        )
                        uws.append(uw)
                    state["uws"] = uws

                def s4():
                    s_ps = psum_s.tile([OUT_F, WAVE], F32, tag="scol")
                    for pl in range(WAVE):
                        p = w0 + pl
                        n = p % NUM
                        ft = work.tile([P, T, OUT_F], BF16, tag="tt")
                        eng = nc.vector if (p % F_DVE_MOD) == 0 else nc.gpsimd
                        eng.tensor_tensor(
                            ft[:, :, :],
                            state["ets"][pl][:, :, :],
                            W_sb[:, n, :, :],
                            op=mult,
                        )
                        for t in range(T):
                            nc.tensor.matmul(
                                s_ps[:, pl : pl + 1],
                                ft[:, t, :],
                                state["uws"][pl][:, t : t + 1],
                                start=(t == 0),
                                stop=(t == T - 1),
                            )
                    nc.vector.tensor_copy(s_next[:, w0 : w0 + WAVE], s_ps[:, :])

                return [s0, s1, s2, s3, s4]

            waves = [make_wave(w * WAVE) for w in range(nwaves)]
            NSTAGE = 5
            for step in range(nwaves + NSTAGE - 1):
                for st in range(NSTAGE - 1, -1, -1):
                    w = step - st
                    if 0 <= w < nwaves:
                        waves[w][st]()
            s_cur = s_next

        for g in range(G):
            squash_group(s_cur, g, V_cur[g], is_final=True)

    nc.compile()
    return nc


_NC_CACHE = {}


def _get_nc(key):
    if key not in _NC_CACHE:
        _NC_CACHE[key] = _build(*key)
    return _NC_CACHE[key]


def _prep(u, weight, bias, c0, routings):
    u = np.ascontiguousarray(np.asarray(u, dtype=np.float32))
    weight = np.ascontiguousarray(
        np.asarray(weight, dtype=np.float32).reshape(weight.shape[-3:])
    )
    bias = np.ascontiguousarray(np.asarray(bias, dtype=np.float32).reshape(bias.shape[-2:]))
    c0 = np.ascontiguousarray(np.asarray(c0, dtype=np.float32).reshape(c0.shape[-2:]))
    routings = int(routings)
    B, NUM, IN_F = u.shape
    OUT_F = weight.shape[-1]
    uniform = bool(np.all(c0 == c0.flat[0]))
    c00 = float(c0.flat[0])
    assert B % N_CORES == 0, f"B={B} not divisible by {N_CORES}"
    B_core = B // N_CORES
    key = (B_core, NUM, IN_F, OUT_F, routings, c00 if uniform else 0.0, uniform)
    return u, weight, bias, c0, routings, B_core, key, uniform


def run_on_hw(u, weight, bias, c0, routings, trace=False):
    """Shard over cores, run SPMD, gather. Returns (out, exec_time_ns|None)."""
    u, weight, bias, c0, routings, B_core, key, uniform = _prep(
        u, weight, bias, c0, routings
    )
    nc = _get_nc(key)
    wbf = weight.astype(ml_dtypes.bfloat16)
    in_maps = []
    for c in range(N_CORES):
        m = {
            "u": u[c * B_core : (c + 1) * B_core],
            "wbf": wbf,
            "bias": bias,
        }
        if not uniform:
            m["c0"] = c0
        in_maps.append(m)
    res = run_bass_kernel_spmd(nc, in_maps, core_ids=list(range(N_CORES)), trace=trace)
    out = np.concatenate([res.results[c]["out"] for c in range(N_CORES)], axis=0)
    return out, res.exec_time_ns


_RUNNER_CACHE = {}


def _get_runner(key):
    """Cached jitted multi-core executable (avoids per-call re-jit)."""
    if key in _RUNNER_CACHE:
        return _RUNNER_CACHE[key]
    import jax
    from jax.sharding import Mesh, PartitionSpec
    from jax.experimental.shard_map import shard_map
    from concourse import bass2jax, mybir as mb

    nc = _get_nc(key)
    bass2jax.install_neuronx_cc_hook()
    part_name = nc.partition_id_tensor.name if nc.partition_id_tensor else None
    in_names, out_names, out_avals, zero_outs = [], [], [], []
    for alloc in nc.m.functions[0].allocations:
        if not isinstance(alloc, mb.MemoryLocationSet):
            continue
        name = alloc.memorylocations[0].name
        if alloc.kind == "ExternalInput":
            if name != part_name:
                in_names.append(name)
        elif alloc.kind == "ExternalOutput":
            out_names.append(name)
            shape = tuple(alloc.tensor_shape)
            dtype = mb.dt.np(alloc.dtype)
            out_avals.append(jax.core.ShapedArray(shape, dtype))
            zero_outs.append(np.zeros(shape, dtype))
    n_params = len(in_names)
    all_names = in_names + out_names
    if part_name is not None:
        all_names = all_names + [part_name]
    donate = tuple(range(n_params, n_params + len(out_names)))

    def _body(*args):
        operands = list(args)
        if part_name is not None:
            operands.append(bass2jax.partition_id_tensor())
        outs = bass2jax._bass_exec_p.bind(
            *operands,
            out_avals=tuple(out_avals),
            in_names=tuple(all_names),
            out_names=tuple(out_names),
            lowering_input_output_aliases=(),
            sim_require_finite=True,
            sim_require_nnan=True,
            nc=nc,
        )
        return tuple(outs)

    devices = jax.devices()[:N_CORES]
    mesh = Mesh(np.asarray(devices), ("core",))
    specs = (PartitionSpec("core"),) * (n_params + len(out_names))
    fn = jax.jit(
        shard_map(
            _body,
            mesh=mesh,
            in_specs=specs,
            out_specs=(PartitionSpec("core"),) * len(out_names),
            check_rep=False,
        ),
        donate_argnums=donate,
        keep_unused=True,
    )
    runner = (fn, in_names, out_names, out_avals, zero_outs)
    _RUNNER_CACHE[key] = runner
    return runner


def run_cached(u, weight, bias, c0, routings):
    """Run via a cached jitted executable. Returns (out, per_call_fn)."""
    u, weight, bias, c0, routings, B_core, key, uniform = _prep(
        u, weight, bias, c0, routings
    )
    fn, in_names, out_names, out_avals, zero_outs = _get_runner(key)
    wbf = weight.astype(ml_dtypes.bfloat16)
    per_core = {
        "u": [u[c * B_core : (c + 1) * B_core] for c in range(N_CORES)],
        "wbf": [wbf] * N_CORES,
        "bias": [bias] * N_CORES,
        "c0": [c0] * N_CORES,
    }
    concat_in = [np.concatenate(per_core[nm], axis=0) for nm in in_names]

    def call():
        zeros = [
            np.zeros((N_CORES * z.shape[0], *z.shape[1:]), z.dtype)
            for z in zero_outs
        ]
        outs = fn(*concat_in, *zeros)
        return np.asarray(outs[0])

    full = call()
    i = out_names.index("out")
    B_total = N_CORES * B_core
    out = full.reshape(N_CORES, B_core, *out_avals[i].shape[1:]).reshape(
        B_total, *out_avals[i].shape[1:]
    )
    return out, call


def kernel(**inputs):
    out, _ = run_on_hw(
        inputs["u"],
        inputs["weight"],
        inputs["bias"],
        inputs["c0"],
        inputs["routings"],
    )
    return out


# revision 29
# speedup vs baseline: 4114.4491x; 1.7152x over previous
"""Trainium2 Bass kernel for CapsuleParall dynamic routing.

Math (per (b, n) pair, u_hat[i,o] = u[i] * W[n][i,o]):
    s_1[o] = sum_i u_hat[i,o] * c0[i,o]
    v_k    = squash(s_k + bias)           (squash over o)
    V_k    = v_1 + ... + v_k              (cumulative; b == u_hat * V)
    c_k    = softmax_o(u_hat[i,o] * V_k[o])
    s_{k+1}[o] = sum_i u_hat[i,o] * c_k[i,o]
    out    = squash(s_routings + bias)

On-chip strategy (layout: i on partitions, free = (chunk, o)):
    e[i,o] = exp(u_hat[i,o] * V[o])  unnormalized (values are small, safe)
    Z[i]   = sum_o e[i,o]            (per-chunk tensor_scalar accum on DVE)
    s[o]   = sum_i (W[i,o]*e[i,o]) * (u[i]/Z[i])
The PE matmul (lhsT = W.e chunk, rhs = (u/Z) column) applies both the u
factor and the softmax normalization during the i-contraction.  Hot-path
tensors are bf16 (DVE 2x/4x modes); accumulations are fp32.

Sharding: data-parallel over batch B across 8 cores (4 batches/core).
"""

import sys

sys.path.insert(0, "/opt/trn_rl_repo")

from contextlib import ExitStack

import numpy as np
import ml_dtypes

import concourse.bass as bass
import concourse.bacc as bacc
import concourse.mybir as mybir
import concourse.tile as tile
from concourse import masks
from concourse.bass_utils import run_bass_kernel_spmd

F32 = mybir.dt.float32
BF16 = mybir.dt.bfloat16
EPS = 1e-5
N_CORES = 8

# engine-split knobs
UHAT_DVE_CHUNKS = 0   # u_hat chunks with index < this go to DVE, rest Pool
F_DVE_MOD = 7         # waves with (w//WAVE % 7) < this run f-mult on DVE
WAVE = 4              # pairs per software-pipeline wave


def _build(B_core, NUM, IN_F, OUT_F, routings, c00, uniform_c0):
    """Build the per-core Bass module."""
    P = 128
    assert IN_F % P == 0
    T = IN_F // P                      # 9 i-chunks
    PAIRS = B_core * NUM               # 64 (b, n) pairs per core
    # squash groups must start at partition 0/32/64/96 (HW AP restriction)
    GP = 32 if (PAIRS % 32 == 0 and PAIRS > 32) else PAIRS
    G = PAIRS // GP
    mult = mybir.AluOpType.mult
    add = mybir.AluOpType.add

    nc = bacc.Bacc("TRN2", target_bir_lowering=False, debug=False)

    u_dram = nc.dram_tensor("u", [B_core, NUM, IN_F], F32, kind="ExternalInput")
    w_dram = nc.dram_tensor("wbf", [NUM, IN_F, OUT_F], BF16, kind="ExternalInput")
    b_dram = nc.dram_tensor("bias", [NUM, OUT_F], F32, kind="ExternalInput")
    if not uniform_c0:
        c0_dram = nc.dram_tensor("c0", [IN_F, OUT_F], F32, kind="ExternalInput")
    out_dram = nc.dram_tensor("out", [B_core, NUM, OUT_F], F32, kind="ExternalOutput")

    def bcast_mid(ap2d, n):
        # [P, F] -> [P, n, F] with the middle dim broadcast (stride 0)
        return bass.AP(ap2d.tensor, ap2d.offset, [ap2d.ap[0], [0, n], ap2d.ap[1]])

    with tile.TileContext(nc) as tc, ExitStack() as ctx:
        const = ctx.enter_context(tc.tile_pool(name="const", bufs=1))
        work = ctx.enter_context(tc.tile_pool(name="work", bufs=3))
        small = ctx.enter_context(tc.tile_pool(name="small", bufs=4))
        sall_pool = ctx.enter_context(tc.tile_pool(name="sall", bufs=2))
        sq_pool = ctx.enter_context(tc.tile_pool(name="sq", bufs=4))
        vflat_pool = ctx.enter_context(tc.tile_pool(name="vflat", bufs=2))
        wave_pool = ctx.enter_context(tc.tile_pool(name="wave", bufs=3))
        psum_s = ctx.enter_context(
            tc.tile_pool(name="psum_s", bufs=2, space=bass.MemorySpace.PSUM)
        )
        psum_vb = ctx.enter_context(
            tc.tile_pool(name="psum_vb", bufs=2, space=bass.MemorySpace.PSUM)
        )
        psum_tr = ctx.enter_context(
            tc.tile_pool(name="psum_tr", bufs=2, space=bass.MemorySpace.PSUM)
        )

        # ---- resident tensors ----
        W_sb = const.tile([P, NUM, T, OUT_F], BF16)      # W[n][i,o], i = t*128+p
        u_nat = const.tile([PAIRS, IN_F], F32)           # natural row layout
        u_sb = const.tile([P, T, PAIRS], F32)            # u columns (i on partitions)
        u_bf = const.tile([P, T, PAIRS], BF16)
        uc_bf = const.tile([P, T, PAIRS], BF16)          # u * c00 (uniform-c0 path)
        bias_all = const.tile([PAIRS, OUT_F], F32)
        ident = const.tile([P, P], F32)
        ones_row = const.tile([1, P], F32)
        if not uniform_c0:
            c0_sb = const.tile([P, T, OUT_F], BF16)

        # ---- loads ----
        w_ap = w_dram.ap()
        # src AP dims: [p(128), n, t, o] in elements of w_dram [NUM, IN_F, OUT_F]
        w_src = bass.AP(
            w_ap.tensor,
            w_ap.offset,
            [[OUT_F, P], [IN_F * OUT_F, NUM], [P * OUT_F, T], [1, OUT_F]],
        )
        nc.sync.dma_start(W_sb[:, :, :, :], w_src)
        nc.sync.dma_start(u_nat[:, :], u_dram.ap().rearrange("b n i -> (b n) i"))
        for b in range(B_core):
            nc.sync.dma_start(bias_all[b * NUM : (b + 1) * NUM, :], b_dram.ap())
        if not uniform_c0:
            c_ap = c0_dram.ap()
            c_src = bass.AP(
                c_ap.tensor, c_ap.offset, [[OUT_F, P], [P * OUT_F, T], [1, OUT_F]]
            )
            c0f = const.tile([P, T, OUT_F], F32)
            nc.sync.dma_start(c0f[:, :, :], c_src)
            nc.vector.tensor_copy(c0_sb[:, :, :], c0f[:, :, :])
        masks.make_identity(nc, ident[:, :])
        nc.vector.memset(ones_row[:, :], 1.0)

        # u_nat [PAIRS, IN_F] -> u_sb [P, T, PAIRS] via PE transposes per chunk
        for t in range(T):
            tr = psum_tr.tile([P, PAIRS], F32, tag="tr")
            nc.tensor.transpose(
                tr[:, :], u_nat[:, t * P : (t + 1) * P], ident[:PAIRS, :PAIRS]
            )
            nc.vector.tensor_copy(u_sb[:, t, :], tr[:, :])
        nc.vector.tensor_copy(u_bf[:, :, :], u_sb[:, :, :])
        nc.vector.tensor_scalar_mul(uc_bf[:, :, :], u_bf[:, :, :], float(c00))

        # ---- phase 1: s_1 for every pair ----
        s_allT = sall_pool.tile([P, PAIRS], F32, tag="sall")
        for p in range(PAIRS):
            n = p % NUM
            s_col = psum_s.tile([OUT_F, 1], F32, tag="scol")
            for t in range(T):
                if uniform_c0:
                    nc.tensor.matmul(
                        s_col[:, :],
                        W_sb[:, n, t, :],
                        uc_bf[:, t, p : p + 1],
                        start=(t == 0),
                        stop=(t == T - 1),
                    )
                else:
                    wc = work.tile([P, OUT_F], BF16, tag="wc0")
                    nc.vector.tensor_tensor(
                        wc[:, :], W_sb[:, n, t, :], c0_sb[:, t, :], op=mult
                    )
                    nc.tensor.matmul(
                        s_col[:, :],
                        wc[:, :],
                        u_bf[:, t, p : p + 1],
                        start=(t == 0),
                        stop=(t == T - 1),
                    )
            nc.vector.tensor_copy(s_allT[:, p : p + 1], s_col[:, :])

        # ---- squash (batched over a group of GP pairs) ----
        def squash_group(s_tile, g, V_prev, is_final):
            r0, r1 = g * GP, (g + 1) * GP
            tr = psum_tr.tile([GP, OUT_F], F32, tag="tr")
            nc.tensor.transpose(tr[:, :], s_tile[:, r0:r1], ident[:, :])
            sb = sq_pool.tile([GP, OUT_F], F32, tag="sb")
            nc.vector.tensor_tensor(sb[:, :], tr[:, :], bias_all[r0:r1, :], op=add)
            sqs = sq_pool.tile([GP, OUT_F], F32, tag="sqs")
            n2 = sq_pool.tile([GP, 1], F32, tag="n2")
            nc.vector.tensor_tensor(sqs[:, :], sb[:, :], sb[:, :], op=mult)
            nc.vector.tensor_reduce(
                n2[:, :], sqs[:, :], axis=mybir.AxisListType.X, op=add
            )
            rt = sq_pool.tile([GP, 1], F32, tag="rt")
            nc.scalar.activation(rt[:, :], n2[:, :], mybir.ActivationFunctionType.Sqrt)
            d1 = sq_pool.tile([GP, 1], F32, tag="d1")
            nc.vector.tensor_scalar_add(d1[:, :], n2[:, :], 1.0)
            d2 = sq_pool.tile([GP, 1], F32, tag="d2")
            nc.vector.tensor_scalar_add(d2[:, :], rt[:, :], EPS)
            den = sq_pool.tile([GP, 1], F32, tag="den")
            nc.vector.tensor_tensor(den[:, :], d1[:, :], d2[:, :], op=mult)
            rden = sq_pool.tile([GP, 1], F32, tag="rden")
            nc.vector.reciprocal(rden[:, :], den[:, :])
            coef = sq_pool.tile([GP, 1], F32, tag="coef")
            nc.vector.tensor_tensor(coef[:, :], n2[:, :], rden[:, :], op=mult)
            v = sq_pool.tile([GP, OUT_F], F32, tag="v")
            nc.vector.tensor_scalar_mul(v[:, :], sb[:, :], coef[:, 0:1])
            if is_final:
                out_rows = out_dram.ap().rearrange("b n o -> (b n) o")
                nc.sync.dma_start(out_rows[r0:r1, :], v[:, :])
                return None
            if V_prev is None:
                V_new = v
            else:
                V_new = sq_pool.tile([GP, OUT_F], F32, tag="V")
                nc.vector.tensor_tensor(V_new[:, :], V_prev[:, :], v[:, :], op=add)
            return V_new

        # ---- routing iterations (software-pipelined waves of WAVE pairs) ----
        V_cur = [None] * G
        s_cur = s_allT
        for k in range(2, routings + 1):
            s_next = sall_pool.tile([P, PAIRS], F32, tag="sall")
            for g in range(G):
                V_cur[g] = squash_group(s_cur, g, V_cur[g], is_final=False)
            nwaves = PAIRS // WAVE

            def make_wave(w0):
                g = w0 // GP
                gl0 = w0 % GP
                n0 = w0 % NUM
                NW = WAVE * OUT_F
                state = {}

                def s0():
                    # V rows -> flat row -> broadcast across partitions (PE)
                    V_flat = vflat_pool.tile([1, NW], F32, tag="vflat")
                    nc.sync.dma_start(V_flat[0:1, :], V_cur[g][gl0 : gl0 + WAVE, :])
                    vb_ps = psum_vb.tile([P, NW], F32, tag="vb")
                    for j in range(0, NW, 512):
                        jn = min(512, NW - j)
                        nc.tensor.matmul(
                            vb_ps[:, j : j + jn],
                            ones_row[:, :],
                            V_flat[0:1, j : j + jn],
                            start=True,
                            stop=True,
                        )
                    vb16 = small.tile([P, WAVE, OUT_F], BF16, tag="vb16")
                    nc.scalar.copy(vb16[:, :, :], vb_ps[:, :])
                    # u_hat for the whole wave in one op: W[n0:n0+W] * u (bcast o)
                    uh = wave_pool.tile([P, WAVE, T, OUT_F], BF16, tag="uhat")
                    for pl in range(WAVE):
                        p = w0 + pl
                        for t in range(T):
                            if (pl * T + t) % 6 == 0:
                                nc.scalar.mul(
                                    uh[:, pl, t, :],
                                    W_sb[:, n0 + pl, t, :],
                                    u_sb[:, t, p : p + 1],
                                )
                            else:
                                nc.gpsimd.tensor_scalar_mul(
                                    uh[:, pl, t, :],
                                    W_sb[:, n0 + pl, t, :],
                                    u_sb[:, t, p : p + 1],
                                )
                    state["vb16"] = vb16
                    state["uh"] = uh

                def s1():
                    # t = u_hat * V  (vb16 broadcast over chunks)
                    tt_ = wave_pool.tile([P, WAVE, T, OUT_F], BF16, tag="tt")
                    vb = state["vb16"][:, :, :]
                    vbb = bass.AP(
                        vb.tensor, vb.offset,
                        [vb.ap[0], [OUT_F, WAVE], [0, T], [1, OUT_F]],
                    )
                    nc.vector.tensor_tensor(
                        tt_[:, :, :, :], state["uh"][:, :, :, :], vbb, op=mult
                    )
                    state["tt"] = tt_

                def s2():
                    et = wave_pool.tile([P, WAVE, T, OUT_F], BF16, tag="e")
                    nc.scalar.activation(
                        et[:, :, :, :],
                        state["tt"][:, :, :, :],
                        mybir.ActivationFunctionType.Exp,
                    )
                    state["et"] = et

                def s3():
                    # Z[pair, chunk] = sum_o e;  uw = u / Z
                    Z = small.tile([P, WAVE, T], F32, tag="Z")
                    et = state["et"]
                    for pl in range(WAVE):
                        for t in range(T):
                            nc.vector.tensor_scalar(
                                et[:, pl, t, :],
                                et[:, pl, t, :],
                                1.0,
                                None,
                                mult,
                                op1=add,
                                accum_out=Z[:, pl, t : t + 1],
                            )
                    wr = small.tile([P, WAVE, T], F32, tag="wr")
                    nc.vector.reciprocal(wr[:, :, :], Z[:, :, :])
                    uw = small.tile([P, WAVE, T], BF16, tag="uw")
                    us = u_sb[:, :, w0 : w0 + WAVE]
                    usb = bass.AP(
                        us.tensor, us.offset, [us.ap[0], [1, WAVE], [PAIRS, T]]
                    )
                    nc.vector.tensor_tensor(uw[:, :, :], wr[:, :, :], usb, op=mult)
                    state["uw"] = uw

                def s4():
                    ft = wave_pool.tile([P, WAVE, T, OUT_F], BF16, tag="tt")
                    eng = nc.vector if (w0 // WAVE) % 7 < F_DVE_MOD else nc.gpsimd
                    eng.tensor_tensor(
                        ft[:, :, :, :],
                        state["et"][:, :, :, :],
                        W_sb[:, n0 : n0 + WAVE, :, :],
                        op=mult,
                    )
                    s_ps = psum_s.tile([OUT_F, WAVE], F32, tag="scol")
                    for pl in range(WAVE):
                        for t in range(T):
                            nc.tensor.matmul(
                                s_ps[:, pl : pl + 1],
                                ft[:, pl, t, :],
                                state["uw"][:, pl, t : t + 1],
                                start=(t == 0),
                                stop=(t == T - 1),
                            )
                    nc.scalar.copy(s_next[:, w0 : w0 + WAVE], s_ps[:, :])

                return [s0, s1, s2, s3, s4]

            waves = [make_wave(w * WAVE) for w in range(nwaves)]
            NSTAGE = 5
            for step in range(nwaves + NSTAGE - 1):
                for st in range(NSTAGE - 1, -1, -1):
                    w = step - st
                    if 0 <= w < nwaves:
                        waves[w][st]()
            s_cur = s_next

        for g in range(G):
            squash_group(s_cur, g, V_cur[g], is_final=True)

    nc.compile()
    return nc


_NC_CACHE = {}


def _get_nc(key):
    if key not in _NC_CACHE:
        _NC_CACHE[key] = _build(*key)
    return _NC_CACHE[key]


def _prep(u, weight, bias, c0, routings):
    u = np.ascontiguousarray(np.asarray(u, dtype=np.float32))
    weight = np.ascontiguousarray(
        np.asarray(weight, dtype=np.float32).reshape(weight.shape[-3:])
    )
    bias = np.ascontiguousarray(np.asarray(bias, dtype=np.float32).reshape(bias.shape[-2:]))
    c0 = np.ascontiguousarray(np.asarray(c0, dtype=np.float32).reshape(c0.shape[-2:]))
    routings = int(routings)
    B, NUM, IN_F = u.shape
    OUT_F = weight.shape[-1]
    uniform = bool(np.all(c0 == c0.flat[0]))
    c00 = float(c0.flat[0])
    assert B % N_CORES == 0, f"B={B} not divisible by {N_CORES}"
    B_core = B // N_CORES
    key = (B_core, NUM, IN_F, OUT_F, routings, c00 if uniform else 0.0, uniform)
    return u, weight, bias, c0, routings, B_core, key, uniform


def run_on_hw(u, weight, bias, c0, routings, trace=False):
    """Shard over cores, run SPMD, gather. Returns (out, exec_time_ns|None)."""
    u, weight, bias, c0, routings, B_core, key, uniform = _prep(
        u, weight, bias, c0, routings
    )
    nc = _get_nc(key)
    wbf = weight.astype(ml_dtypes.bfloat16)
    in_maps = []
    for c in range(N_CORES):
        m = {
            "u": u[c * B_core : (c + 1) * B_core],
            "wbf": wbf,
            "bias": bias,
        }
        if not uniform:
            m["c0"] = c0
        in_maps.append(m)
    res = run_bass_kernel_spmd(nc, in_maps, core_ids=list(range(N_CORES)), trace=trace)
    out = np.concatenate([res.results[c]["out"] for c in range(N_CORES)], axis=0)
    return out, res.exec_time_ns


_RUNNER_CACHE = {}


def _get_runner(key):
    """Cached jitted multi-core executable (avoids per-call re-jit)."""
    if key in _RUNNER_CACHE:
        return _RUNNER_CACHE[key]
    import jax
    from jax.sharding import Mesh, PartitionSpec
    from jax.experimental.shard_map import shard_map
    from concourse import bass2jax, mybir as mb

    nc = _get_nc(key)
    bass2jax.install_neuronx_cc_hook()
    part_name = nc.partition_id_tensor.name if nc.partition_id_tensor else None
    in_names, out_names, out_avals, zero_outs = [], [], [], []
    for alloc in nc.m.functions[0].allocations:
        if not isinstance(alloc, mb.MemoryLocationSet):
            continue
        name = alloc.memorylocations[0].name
        if alloc.kind == "ExternalInput":
            if name != part_name:
                in_names.append(name)
        elif alloc.kind == "ExternalOutput":
            out_names.append(name)
            shape = tuple(alloc.tensor_shape)
            dtype = mb.dt.np(alloc.dtype)
            out_avals.append(jax.core.ShapedArray(shape, dtype))
            zero_outs.append(np.zeros(shape, dtype))
    n_params = len(in_names)
    all_names = in_names + out_names
    if part_name is not None:
        all_names = all_names + [part_name]
    donate = tuple(range(n_params, n_params + len(out_names)))

    def _body(*args):
        operands = list(args)
        if part_name is not None:
            operands.append(bass2jax.partition_id_tensor())
        outs = bass2jax._bass_exec_p.bind(
            *operands,
            out_avals=tuple(out_avals),
            in_names=tuple(all_names),
            out_names=tuple(out_names),
            lowering_input_output_aliases=(),
            sim_require_finite=True,
            sim_require_nnan=True,
            nc=nc,
        )
        return tuple(outs)

    devices = jax.devices()[:N_CORES]
    mesh = Mesh(np.asarray(devices), ("core",))
    specs = (PartitionSpec("core"),) * (n_params + len(out_names))
    fn = jax.jit(
        shard_map(
            _body,
            mesh=mesh,
            in_specs=specs,
            out_specs=(PartitionSpec("core"),) * len(out_names),
            check_rep=False,
        ),
        donate_argnums=donate,
        keep_unused=True,
    )
    runner = (fn, in_names, out_names, out_avals, zero_outs)
    _RUNNER_CACHE[key] = runner
    return runner


def run_cached(u, weight, bias, c0, routings):
    """Run via a cached jitted executable. Returns (out, per_call_fn)."""
    u, weight, bias, c0, routings, B_core, key, uniform = _prep(
        u, weight, bias, c0, routings
    )
    fn, in_names, out_names, out_avals, zero_outs = _get_runner(key)
    wbf = weight.astype(ml_dtypes.bfloat16)
    per_core = {
        "u": [u[c * B_core : (c + 1) * B_core] for c in range(N_CORES)],
        "wbf": [wbf] * N_CORES,
        "bias": [bias] * N_CORES,
        "c0": [c0] * N_CORES,
    }
    concat_in = [np.concatenate(per_core[nm], axis=0) for nm in in_names]

    def call():
        zeros = [
            np.zeros((N_CORES * z.shape[0], *z.shape[1:]), z.dtype)
            for z in zero_outs
        ]
        outs = fn(*concat_in, *zeros)
        return np.asarray(outs[0])

    full = call()
    i = out_names.index("out")
    B_total = N_CORES * B_core
    out = full.reshape(N_CORES, B_core, *out_avals[i].shape[1:]).reshape(
        B_total, *out_avals[i].shape[1:]
    )
    return out, call


def kernel(**inputs):
    out, _ = run_on_hw(
        inputs["u"],
        inputs["weight"],
        inputs["bias"],
        inputs["c0"],
        inputs["routings"],
    )
    return out


# revision 33
# speedup vs baseline: 4156.2928x; 1.0102x over previous
"""Trainium2 Bass kernel for CapsuleParall dynamic routing.

Math (per (b, n) pair, u_hat[i,o] = u[i] * W[n][i,o]):
    s_1[o] = sum_i u_hat[i,o] * c0[i,o]
    v_k    = squash(s_k + bias)           (squash over o)
    V_k    = v_1 + ... + v_k              (cumulative; b == u_hat * V)
    c_k    = softmax_o(u_hat[i,o] * V_k[o])
    s_{k+1}[o] = sum_i u_hat[i,o] * c_k[i,o]
    out    = squash(s_routings + bias)

On-chip strategy (layout: i on partitions, free = (chunk, o)):
    e[i,o] = exp(u_hat[i,o] * V[o])  unnormalized (values are small, safe)
    Z[i]   = sum_o e[i,o]            (per-chunk tensor_scalar accum on DVE)
    s[o]   = sum_i (W[i,o]*e[i,o]) * (u[i]/Z[i])
The PE matmul (lhsT = W.e chunk, rhs = (u/Z) column) applies both the u
factor and the softmax normalization during the i-contraction.  Hot-path
tensors are bf16 (DVE 2x/4x modes); accumulations are fp32.

Sharding: data-parallel over batch B across 8 cores (4 batches/core).
"""

import sys

sys.path.insert(0, "/opt/trn_rl_repo")

from contextlib import ExitStack

import numpy as np
import ml_dtypes

import concourse.bass as bass
import concourse.bacc as bacc
import concourse.mybir as mybir
import concourse.tile as tile
from concourse import masks
from concourse.bass_utils import run_bass_kernel_spmd

F32 = mybir.dt.float32
BF16 = mybir.dt.bfloat16
EPS = 1e-5
N_CORES = 8

# engine-split knobs
UHAT_DVE_CHUNKS = 0   # u_hat chunks with index < this go to DVE, rest Pool
F_DVE_MOD = 7         # waves with (w//WAVE % 7) < this run f-mult on DVE
WAVE = 4              # pairs per software-pipeline wave


def _build(B_core, NUM, IN_F, OUT_F, routings, c00, uniform_c0):
    """Build the per-core Bass module."""
    P = 128
    assert IN_F % P == 0
    T = IN_F // P                      # 9 i-chunks
    PAIRS = B_core * NUM               # 64 (b, n) pairs per core
    # squash groups must start at partition 0/32/64/96 (HW AP restriction)
    GP = 32 if (PAIRS % 32 == 0 and PAIRS > 32) else PAIRS
    G = PAIRS // GP
    mult = mybir.AluOpType.mult
    add = mybir.AluOpType.add

    nc = bacc.Bacc("TRN2", target_bir_lowering=False, debug=False)

    u_dram = nc.dram_tensor("u", [B_core, NUM, IN_F], F32, kind="ExternalInput")
    w_dram = nc.dram_tensor("wbf", [NUM, IN_F, OUT_F], BF16, kind="ExternalInput")
    b_dram = nc.dram_tensor("bias", [NUM, OUT_F], F32, kind="ExternalInput")
    if not uniform_c0:
        c0_dram = nc.dram_tensor("c0", [IN_F, OUT_F], F32, kind="ExternalInput")
    out_dram = nc.dram_tensor("out", [B_core, NUM, OUT_F], F32, kind="ExternalOutput")

    def bcast_mid(ap2d, n):
        # [P, F] -> [P, n, F] with the middle dim broadcast (stride 0)
        return bass.AP(ap2d.tensor, ap2d.offset, [ap2d.ap[0], [0, n], ap2d.ap[1]])

    with tile.TileContext(nc) as tc, ExitStack() as ctx:
        const = ctx.enter_context(tc.tile_pool(name="const", bufs=1))
        work = ctx.enter_context(tc.tile_pool(name="work", bufs=3))
        small = ctx.enter_context(tc.tile_pool(name="small", bufs=6))
        sall_pool = ctx.enter_context(tc.tile_pool(name="sall", bufs=2))
        sq_pool = ctx.enter_context(tc.tile_pool(name="sq", bufs=4))
        vflat_pool = ctx.enter_context(tc.tile_pool(name="vflat", bufs=2))
        wave_pool = ctx.enter_context(tc.tile_pool(name="wave", bufs=4))
        psum_s = ctx.enter_context(
            tc.tile_pool(name="psum_s", bufs=2, space=bass.MemorySpace.PSUM)
        )
        psum_vb = ctx.enter_context(
            tc.tile_pool(name="psum_vb", bufs=2, space=bass.MemorySpace.PSUM)
        )
        psum_tr = ctx.enter_context(
            tc.tile_pool(name="psum_tr", bufs=2, space=bass.MemorySpace.PSUM)
        )

        # ---- resident tensors ----
        W_sb = const.tile([P, NUM, T, OUT_F], BF16)      # W[n][i,o], i = t*128+p
        u_nat = const.tile([PAIRS, IN_F], F32)           # natural row layout
        u_sb = const.tile([P, T, PAIRS], F32)            # u columns (i on partitions)
        u_bf = const.tile([P, T, PAIRS], BF16)
        uc_bf = const.tile([P, T, PAIRS], BF16)          # u * c00 (uniform-c0 path)
        bias_all = const.tile([PAIRS, OUT_F], F32)
        ident = const.tile([P, P], F32)
        ones_row = const.tile([1, P], F32)
        if not uniform_c0:
            c0_sb = const.tile([P, T, OUT_F], BF16)

        # ---- loads ----
        w_ap = w_dram.ap()
        # src AP dims: [p(128), n, t, o] in elements of w_dram [NUM, IN_F, OUT_F]
        w_src = bass.AP(
            w_ap.tensor,
            w_ap.offset,
            [[OUT_F, P], [IN_F * OUT_F, NUM], [P * OUT_F, T], [1, OUT_F]],
        )
        nc.sync.dma_start(u_nat[:, :], u_dram.ap().rearrange("b n i -> (b n) i"))
        for b in range(B_core):
            nc.sync.dma_start(bias_all[b * NUM : (b + 1) * NUM, :], b_dram.ap())
        for n_ in range(NUM):
            w_n = bass.AP(
                w_ap.tensor,
                w_ap.offset + n_ * IN_F * OUT_F,
                [[OUT_F, P], [P * OUT_F, T], [1, OUT_F]],
            )
            nc.sync.dma_start(W_sb[:, n_, :, :], w_n)
        if not uniform_c0:
            c_ap = c0_dram.ap()
            c_src = bass.AP(
                c_ap.tensor, c_ap.offset, [[OUT_F, P], [P * OUT_F, T], [1, OUT_F]]
            )
            c0f = const.tile([P, T, OUT_F], F32)
            nc.sync.dma_start(c0f[:, :, :], c_src)
            nc.vector.tensor_copy(c0_sb[:, :, :], c0f[:, :, :])
        masks.make_identity(nc, ident[:, :])
        nc.vector.memset(ones_row[:, :], 1.0)

        # u_nat [PAIRS, IN_F] -> u_sb [P, T, PAIRS] via PE transposes per chunk
        for t in range(T):
            tr = psum_tr.tile([P, PAIRS], F32, tag="tr")
            nc.tensor.transpose(
                tr[:, :], u_nat[:, t * P : (t + 1) * P], ident[:PAIRS, :PAIRS]
            )
            nc.vector.tensor_copy(u_sb[:, t, :], tr[:, :])
        nc.vector.tensor_copy(u_bf[:, :, :], u_sb[:, :, :])
        nc.vector.tensor_scalar_mul(uc_bf[:, :, :], u_bf[:, :, :], float(c00))

        # ---- phase 1: s_1 for every pair ----
        s_allT = sall_pool.tile([P, PAIRS], F32, tag="sall")
        for p in range(PAIRS):
            n = p % NUM
            s_col = psum_s.tile([OUT_F, 1], F32, tag="scol")
            for t in range(T):
                if uniform_c0:
                    nc.tensor.matmul(
                        s_col[:, :],
                        W_sb[:, n, t, :],
                        uc_bf[:, t, p : p + 1],
                        start=(t == 0),
                        stop=(t == T - 1),
                    )
                else:
                    wc = work.tile([P, OUT_F], BF16, tag="wc0")
                    nc.vector.tensor_tensor(
                        wc[:, :], W_sb[:, n, t, :], c0_sb[:, t, :], op=mult
                    )
                    nc.tensor.matmul(
                        s_col[:, :],
                        wc[:, :],
                        u_bf[:, t, p : p + 1],
                        start=(t == 0),
                        stop=(t == T - 1),
                    )
            nc.vector.tensor_copy(s_allT[:, p : p + 1], s_col[:, :])

        # ---- squash (batched over a group of GP pairs) ----
        def squash_group(s_tile, g, V_prev, is_final):
            r0, r1 = g * GP, (g + 1) * GP
            tr = psum_tr.tile([GP, OUT_F], F32, tag="tr")
            nc.tensor.transpose(tr[:, :], s_tile[:, r0:r1], ident[:, :])
            sb = sq_pool.tile([GP, OUT_F], F32, tag="sb")
            nc.vector.tensor_tensor(sb[:, :], tr[:, :], bias_all[r0:r1, :], op=add)
            sqs = sq_pool.tile([GP, OUT_F], F32, tag="sqs")
            n2 = sq_pool.tile([GP, 1], F32, tag="n2")
            nc.vector.tensor_tensor(sqs[:, :], sb[:, :], sb[:, :], op=mult)
            nc.vector.tensor_reduce(
                n2[:, :], sqs[:, :], axis=mybir.AxisListType.X, op=add
            )
            rt = sq_pool.tile([GP, 1], F32, tag="rt")
            nc.scalar.activation(rt[:, :], n2[:, :], mybir.ActivationFunctionType.Sqrt)
            d1 = sq_pool.tile([GP, 1], F32, tag="d1")
            nc.vector.tensor_scalar_add(d1[:, :], n2[:, :], 1.0)
            d2 = sq_pool.tile([GP, 1], F32, tag="d2")
            nc.vector.tensor_scalar_add(d2[:, :], rt[:, :], EPS)
            den = sq_pool.tile([GP, 1], F32, tag="den")
            nc.vector.tensor_tensor(den[:, :], d1[:, :], d2[:, :], op=mult)
            rden = sq_pool.tile([GP, 1], F32, tag="rden")
            nc.vector.reciprocal(rden[:, :], den[:, :])
            coef = sq_pool.tile([GP, 1], F32, tag="coef")
            nc.vector.tensor_tensor(coef[:, :], n2[:, :], rden[:, :], op=mult)
            v = sq_pool.tile([GP, OUT_F], F32, tag="v")
            nc.vector.tensor_scalar_mul(v[:, :], sb[:, :], coef[:, 0:1])
            if is_final:
                out_rows = out_dram.ap().rearrange("b n o -> (b n) o")
                nc.sync.dma_start(out_rows[r0:r1, :], v[:, :])
                return None
            if V_prev is None:
                V_new = v
            else:
                V_new = sq_pool.tile([GP, OUT_F], F32, tag="V")
                nc.vector.tensor_tensor(V_new[:, :], V_prev[:, :], v[:, :], op=add)
            return V_new

        # ---- routing iterations (software-pipelined waves of WAVE pairs) ----
        V_cur = [None] * G
        s_cur = s_allT
        for k in range(2, routings + 1):
            s_next = sall_pool.tile([P, PAIRS], F32, tag="sall")
            for g in range(G):
                V_cur[g] = squash_group(s_cur, g, V_cur[g], is_final=False)
            nwaves = PAIRS // WAVE

            def make_wave(w0):
                g = w0 // GP
                gl0 = w0 % GP
                n0 = w0 % NUM
                NW = WAVE * OUT_F
                state = {}

                def s0():
                    # V rows -> flat row -> broadcast across partitions (PE)
                    V_flat = vflat_pool.tile([1, NW], F32, tag="vflat")
                    nc.sync.dma_start(V_flat[0:1, :], V_cur[g][gl0 : gl0 + WAVE, :])
                    vb_ps = psum_vb.tile([P, NW], F32, tag="vb")
                    for j in range(0, NW, 512):
                        jn = min(512, NW - j)
                        nc.tensor.matmul(
                            vb_ps[:, j : j + jn],
                            ones_row[:, :],
                            V_flat[0:1, j : j + jn],
                            start=True,
                            stop=True,
                        )
                    vb16 = small.tile([P, WAVE, OUT_F], BF16, tag="vb16")
                    nc.scalar.copy(vb16[:, :, :], vb_ps[:, :])
                    # u_hat for the whole wave in one op: W[n0:n0+W] * u (bcast o)
                    uh = wave_pool.tile([P, WAVE, T, OUT_F], BF16, tag="uhat")
                    for pl in range(WAVE):
                        p = w0 + pl
                        for t in range(T):
                            if (pl * T + t) % 6 == 0:
                                nc.scalar.mul(
                                    uh[:, pl, t, :],
                                    W_sb[:, n0 + pl, t, :],
                                    u_sb[:, t, p : p + 1],
                                )
                            else:
                                nc.gpsimd.tensor_scalar_mul(
                                    uh[:, pl, t, :],
                                    W_sb[:, n0 + pl, t, :],
                                    u_sb[:, t, p : p + 1],
                                )
                    state["vb16"] = vb16
                    state["uh"] = uh

                def s1():
                    # t = u_hat * V  (vb16 broadcast over chunks)
                    tt_ = wave_pool.tile([P, WAVE, T, OUT_F], BF16, tag="tt")
                    vb = state["vb16"][:, :, :]
                    vbb = bass.AP(
                        vb.tensor, vb.offset,
                        [vb.ap[0], [OUT_F, WAVE], [0, T], [1, OUT_F]],
                    )
                    nc.vector.tensor_tensor(
                        tt_[:, :, :, :], state["uh"][:, :, :, :], vbb, op=mult
                    )
                    state["tt"] = tt_

                def s2():
                    et = wave_pool.tile([P, WAVE, T, OUT_F], BF16, tag="e")
                    nc.scalar.activation(
                        et[:, :, :, :],
                        state["tt"][:, :, :, :],
                        mybir.ActivationFunctionType.Exp,
                    )
                    state["et"] = et

                def s3():
                    # Z[pair, chunk] = sum_o e;  uw = u / Z
                    Z = small.tile([P, WAVE, T], F32, tag="Z")
                    et = state["et"]
                    for pl in range(WAVE):
                        for t in range(T):
                            nc.vector.tensor_scalar(
                                et[:, pl, t, :],
                                et[:, pl, t, :],
                                1.0,
                                None,
                                mult,
                                op1=add,
                                accum_out=Z[:, pl, t : t + 1],
                            )
                    wr = small.tile([P, WAVE, T], F32, tag="wr")
                    nc.vector.reciprocal(wr[:, :, :], Z[:, :, :])
                    uw = small.tile([P, WAVE, T], BF16, tag="uw")
                    us = u_sb[:, :, w0 : w0 + WAVE]
                    usb = bass.AP(
                        us.tensor, us.offset, [us.ap[0], [1, WAVE], [PAIRS, T]]
                    )
                    nc.vector.tensor_tensor(uw[:, :, :], wr[:, :, :], usb, op=mult)
                    state["uw"] = uw

                def s4():
                    ft = wave_pool.tile([P, WAVE, T, OUT_F], BF16, tag="tt")
                    eng = nc.vector if (w0 // WAVE) % 7 < F_DVE_MOD else nc.gpsimd
                    eng.tensor_tensor(
                        ft[:, :, :, :],
                        state["et"][:, :, :, :],
                        W_sb[:, n0 : n0 + WAVE, :, :],
                        op=mult,
                    )
                    s_ps = psum_s.tile([OUT_F, WAVE], F32, tag="scol")
                    for pl in range(WAVE):
                        for t in range(T):
                            nc.tensor.matmul(
                                s_ps[:, pl : pl + 1],
                                ft[:, pl, t, :],
                                state["uw"][:, pl, t : t + 1],
                                start=(t == 0),
                                stop=(t == T - 1),
                            )
                    nc.scalar.copy(s_next[:, w0 : w0 + WAVE], s_ps[:, :])

                return [s0, s1, s2, s3, s4]

            waves = [make_wave(w * WAVE) for w in range(nwaves)]
            NSTAGE = 5
            for step in range(nwaves + NSTAGE - 1):
                for st in range(NSTAGE - 1, -1, -1):
                    w = step - st
                    if 0 <= w < nwaves:
                        waves[w][st]()
            s_cur = s_next

        for g in range(G):
            squash_group(s_cur, g, V_cur[g], is_final=True)

    nc.compile()
    return nc


_NC_CACHE = {}


def _get_nc(key):
    if key not in _NC_CACHE:
        _NC_CACHE[key] = _build(*key)
    return _NC_CACHE[key]


def _prep(u, weight, bias, c0, routings):
    u = np.ascontiguousarray(np.asarray(u, dtype=np.float32))
    weight = np.ascontiguousarray(
        np.asarray(weight, dtype=np.float32).reshape(weight.shape[-3:])
    )
    bias = np.ascontiguousarray(np.asarray(bias, dtype=np.float32).reshape(bias.shape[-2:]))
    c0 = np.ascontiguousarray(np.asarray(c0, dtype=np.float32).reshape(c0.shape[-2:]))
    routings = int(routings)
    B, NUM, IN_F = u.shape
    OUT_F = weight.shape[-1]
    uniform = bool(np.all(c0 == c0.flat[0]))
    c00 = float(c0.flat[0])
    assert B % N_CORES == 0, f"B={B} not divisible by {N_CORES}"
    B_core = B // N_CORES
    key = (B_core, NUM, IN_F, OUT_F, routings, c00 if uniform else 0.0, uniform)
    return u, weight, bias, c0, routings, B_core, key, uniform


def run_on_hw(u, weight, bias, c0, routings, trace=False):
    """Shard over cores, run SPMD, gather. Returns (out, exec_time_ns|None)."""
    u, weight, bias, c0, routings, B_core, key, uniform = _prep(
        u, weight, bias, c0, routings
    )
    nc = _get_nc(key)
    wbf = weight.astype(ml_dtypes.bfloat16)
    in_maps = []
    for c in range(N_CORES):
        m = {
            "u": u[c * B_core : (c + 1) * B_core],
            "wbf": wbf,
            "bias": bias,
        }
        if not uniform:
            m["c0"] = c0
        in_maps.append(m)
    res = run_bass_kernel_spmd(nc, in_maps, core_ids=list(range(N_CORES)), trace=trace)
    out = np.concatenate([res.results[c]["out"] for c in range(N_CORES)], axis=0)
    return out, res.exec_time_ns


_RUNNER_CACHE = {}


def _get_runner(key):
    """Cached jitted multi-core executable (avoids per-call re-jit)."""
    if key in _RUNNER_CACHE:
        return _RUNNER_CACHE[key]
    import jax
    from jax.sharding import Mesh, PartitionSpec
    from jax.experimental.shard_map import shard_map
    from concourse import bass2jax, mybir as mb

    nc = _get_nc(key)
    bass2jax.install_neuronx_cc_hook()
    part_name = nc.partition_id_tensor.name if nc.partition_id_tensor else None
    in_names, out_names, out_avals, zero_outs = [], [], [], []
    for alloc in nc.m.functions[0].allocations:
        if not isinstance(alloc, mb.MemoryLocationSet):
            continue
        name = alloc.memorylocations[0].name
        if alloc.kind == "ExternalInput":
            if name != part_name:
                in_names.append(name)
        elif alloc.kind == "ExternalOutput":
            out_names.append(name)
            shape = tuple(alloc.tensor_shape)
            dtype = mb.dt.np(alloc.dtype)
            out_avals.append(jax.core.ShapedArray(shape, dtype))
            zero_outs.append(np.zeros(shape, dtype))
    n_params = len(in_names)
    all_names = in_names + out_names
    if part_name is not None:
        all_names = all_names + [part_name]
    donate = tuple(range(n_params, n_params + len(out_names)))

    def _body(*args):
        operands = list(args)
        if part_name is not None:
            operands.append(bass2jax.partition_id_tensor())
        outs = bass2jax._bass_exec_p.bind(
            *operands,
            out_avals=tuple(out_avals),
            in_names=tuple(all_names),
            out_names=tuple(out_names),
            lowering_input_output_aliases=(),
            sim_require_finite=True,
            sim_require_nnan=True,
            nc=nc,
        )
        return tuple(outs)

    devices = jax.devices()[:N_CORES]
    mesh = Mesh(np.asarray(devices), ("core",))
    specs = (PartitionSpec("core"),) * (n_params + len(out_names))
    fn = jax.jit(
        shard_map(
            _body,
            mesh=mesh,
            in_specs=specs,
            out_specs=(PartitionSpec("core"),) * len(out_names),
            check_rep=False,
        ),
        donate_argnums=donate,
        keep_unused=True,
    )
    runner = (fn, in_names, out_names, out_avals, zero_outs)
    _RUNNER_CACHE[key] = runner
    return runner


def run_cached(u, weight, bias, c0, routings):
    """Run via a cached jitted executable. Returns (out, per_call_fn)."""
    u, weight, bias, c0, routings, B_core, key, uniform = _prep(
        u, weight, bias, c0, routings
    )
    fn, in_names, out_names, out_avals, zero_outs = _get_runner(key)
    wbf = weight.astype(ml_dtypes.bfloat16)
    per_core = {
        "u": [u[c * B_core : (c + 1) * B_core] for c in range(N_CORES)],
        "wbf": [wbf] * N_CORES,
        "bias": [bias] * N_CORES,
        "c0": [c0] * N_CORES,
    }
    concat_in = [np.concatenate(per_core[nm], axis=0) for nm in in_names]

    def call():
        zeros = [
            np.zeros((N_CORES * z.shape[0], *z.shape[1:]), z.dtype)
            for z in zero_outs
        ]
        outs = fn(*concat_in, *zeros)
        return np.asarray(outs[0])

    full = call()
    i = out_names.index("out")
    B_total = N_CORES * B_core
    out = full.reshape(N_CORES, B_core, *out_avals[i].shape[1:]).reshape(
        B_total, *out_avals[i].shape[1:]
    )
    return out, call


def kernel(**inputs):
    out, _ = run_on_hw(
        inputs["u"],
        inputs["weight"],
        inputs["bias"],
        inputs["c0"],
        inputs["routings"],
    )
    return out


# revision 35
# speedup vs baseline: 4182.4578x; 1.0063x over previous
"""Trainium2 Bass kernel for CapsuleParall dynamic routing.

Math (per (b, n) pair, u_hat[i,o] = u[i] * W[n][i,o]):
    s_1[o] = sum_i u_hat[i,o] * c0[i,o]
    v_k    = squash(s_k + bias)           (squash over o)
    V_k    = v_1 + ... + v_k              (cumulative; b == u_hat * V)
    c_k    = softmax_o(u_hat[i,o] * V_k[o])
    s_{k+1}[o] = sum_i u_hat[i,o] * c_k[i,o]
    out    = squash(s_routings + bias)

On-chip strategy (layout: i on partitions, free = (chunk, o)):
    e[i,o] = exp(u_hat[i,o] * V[o])  unnormalized (values are small, safe)
    Z[i]   = sum_o e[i,o]            (per-chunk tensor_scalar accum on DVE)
    s[o]   = sum_i (W[i,o]*e[i,o]) * (u[i]/Z[i])
The PE matmul (lhsT = W.e chunk, rhs = (u/Z) column) applies both the u
factor and the softmax normalization during the i-contraction.  Hot-path
tensors are bf16 (DVE 2x/4x modes); accumulations are fp32.

Sharding: data-parallel over batch B across 8 cores (4 batches/core).
"""

import sys

sys.path.insert(0, "/opt/trn_rl_repo")

from contextlib import ExitStack

import numpy as np
import ml_dtypes

import concourse.bass as bass
import concourse.bacc as bacc
import concourse.mybir as mybir
import concourse.tile as tile
from concourse import masks
from concourse.bass_utils import run_bass_kernel_spmd

F32 = mybir.dt.float32
BF16 = mybir.dt.bfloat16
EPS = 1e-5
N_CORES = 8

# engine-split knobs
UHAT_DVE_CHUNKS = 0   # u_hat chunks with index < this go to DVE, rest Pool
F_DVE_MOD = 7         # waves with (w//WAVE % 7) < this run f-mult on DVE
WAVE = 4              # pairs per software-pipeline wave


def _build(B_core, NUM, IN_F, OUT_F, routings, c00, uniform_c0):
    """Build the per-core Bass module."""
    P = 128
    assert IN_F % P == 0
    T = IN_F // P                      # 9 i-chunks
    PAIRS = B_core * NUM               # 64 (b, n) pairs per core
    # squash groups must start at partition 0/32/64/96 (HW AP restriction)
    GP = 32 if (PAIRS % 32 == 0 and PAIRS > 32) else PAIRS
    G = PAIRS // GP
    mult = mybir.AluOpType.mult
    add = mybir.AluOpType.add

    nc = bacc.Bacc("TRN2", target_bir_lowering=False, debug=False)

    u_dram = nc.dram_tensor("u", [B_core, NUM, IN_F], F32, kind="ExternalInput")
    w_dram = nc.dram_tensor("wbf", [NUM, IN_F, OUT_F], BF16, kind="ExternalInput")
    b_dram = nc.dram_tensor("bias", [NUM, OUT_F], F32, kind="ExternalInput")
    if not uniform_c0:
        c0_dram = nc.dram_tensor("c0", [IN_F, OUT_F], F32, kind="ExternalInput")
    out_dram = nc.dram_tensor("out", [B_core, NUM, OUT_F], F32, kind="ExternalOutput")

    def bcast_mid(ap2d, n):
        # [P, F] -> [P, n, F] with the middle dim broadcast (stride 0)
        return bass.AP(ap2d.tensor, ap2d.offset, [ap2d.ap[0], [0, n], ap2d.ap[1]])

    with tile.TileContext(nc) as tc, ExitStack() as ctx:
        const = ctx.enter_context(tc.tile_pool(name="const", bufs=1))
        work = ctx.enter_context(tc.tile_pool(name="work", bufs=3))
        small = ctx.enter_context(tc.tile_pool(name="small", bufs=6))
        sall_pool = ctx.enter_context(tc.tile_pool(name="sall", bufs=2))
        sq_pool = ctx.enter_context(tc.tile_pool(name="sq", bufs=4))
        vflat_pool = ctx.enter_context(tc.tile_pool(name="vflat", bufs=2))
        wave_pool = ctx.enter_context(tc.tile_pool(name="wave", bufs=4))
        psum_s = ctx.enter_context(
            tc.tile_pool(name="psum_s", bufs=2, space=bass.MemorySpace.PSUM)
        )
        psum_vb = ctx.enter_context(
            tc.tile_pool(name="psum_vb", bufs=2, space=bass.MemorySpace.PSUM)
        )
        psum_tr = ctx.enter_context(
            tc.tile_pool(name="psum_tr", bufs=2, space=bass.MemorySpace.PSUM)
        )

        # ---- resident tensors ----
        W_sb = const.tile([P, NUM, T, OUT_F], BF16)      # W[n][i,o], i = t*128+p
        u_nat = const.tile([PAIRS, IN_F], F32)           # natural row layout
        u_sb = const.tile([P, T, PAIRS], F32)            # u columns (i on partitions)
        u_bf = const.tile([P, T, PAIRS], BF16)
        uc_bf = const.tile([P, T, PAIRS], BF16)          # u * c00 (uniform-c0 path)
        bias_all = const.tile([PAIRS, OUT_F], F32)
        ident = const.tile([P, P], F32)
        ones_row = const.tile([1, P], F32)
        if not uniform_c0:
            c0_sb = const.tile([P, T, OUT_F], BF16)

        # ---- loads ----
        w_ap = w_dram.ap()
        # src AP dims: [p(128), n, t, o] in elements of w_dram [NUM, IN_F, OUT_F]
        w_src = bass.AP(
            w_ap.tensor,
            w_ap.offset,
            [[OUT_F, P], [IN_F * OUT_F, NUM], [P * OUT_F, T], [1, OUT_F]],
        )
        nc.sync.dma_start(u_nat[:, :], u_dram.ap().rearrange("b n i -> (b n) i"))
        for b in range(B_core):
            nc.sync.dma_start(bias_all[b * NUM : (b + 1) * NUM, :], b_dram.ap())
        dma_engs = [nc.sync, nc.scalar, nc.gpsimd]
        for n_ in range(NUM):
            w_n = bass.AP(
                w_ap.tensor,
                w_ap.offset + n_ * IN_F * OUT_F,
                [[OUT_F, P], [P * OUT_F, T], [1, OUT_F]],
            )
            dma_engs[n_ % len(dma_engs)].dma_start(W_sb[:, n_, :, :], w_n)
        if not uniform_c0:
            c_ap = c0_dram.ap()
            c_src = bass.AP(
                c_ap.tensor, c_ap.offset, [[OUT_F, P], [P * OUT_F, T], [1, OUT_F]]
            )
            c0f = const.tile([P, T, OUT_F], F32)
            nc.sync.dma_start(c0f[:, :, :], c_src)
            nc.vector.tensor_copy(c0_sb[:, :, :], c0f[:, :, :])
        masks.make_identity(nc, ident[:, :])
        nc.vector.memset(ones_row[:, :], 1.0)

        # u_nat [PAIRS, IN_F] -> u_sb [P, T, PAIRS] via PE transposes per chunk
        for t in range(T):
            tr = psum_tr.tile([P, PAIRS], F32, tag="tr")
            nc.tensor.transpose(
                tr[:, :], u_nat[:, t * P : (t + 1) * P], ident[:PAIRS, :PAIRS]
            )
            nc.vector.tensor_copy(u_sb[:, t, :], tr[:, :])
        nc.vector.tensor_copy(u_bf[:, :, :], u_sb[:, :, :])
        nc.vector.tensor_scalar_mul(uc_bf[:, :, :], u_bf[:, :, :], float(c00))

        # ---- phase 1: s_1 for every pair ----
        s_allT = sall_pool.tile([P, PAIRS], F32, tag="sall")
        for p in range(PAIRS):
            n = p % NUM
            s_col = psum_s.tile([OUT_F, 1], F32, tag="scol")
            for t in range(T):
                if uniform_c0:
                    nc.tensor.matmul(
                        s_col[:, :],
                        W_sb[:, n, t, :],
                        uc_bf[:, t, p : p + 1],
                        start=(t == 0),
                        stop=(t == T - 1),
                    )
                else:
                    wc = work.tile([P, OUT_F], BF16, tag="wc0")
                    nc.vector.tensor_tensor(
                        wc[:, :], W_sb[:, n, t, :], c0_sb[:, t, :], op=mult
                    )
                    nc.tensor.matmul(
                        s_col[:, :],
                        wc[:, :],
                        u_bf[:, t, p : p + 1],
                        start=(t == 0),
                        stop=(t == T - 1),
                    )
            nc.vector.tensor_copy(s_allT[:, p : p + 1], s_col[:, :])

        # ---- squash (batched over a group of GP pairs) ----
        def squash_group(s_tile, g, V_prev, is_final):
            r0, r1 = g * GP, (g + 1) * GP
            tr = psum_tr.tile([GP, OUT_F], F32, tag="tr")
            nc.tensor.transpose(tr[:, :], s_tile[:, r0:r1], ident[:, :])
            sb = sq_pool.tile([GP, OUT_F], F32, tag="sb")
            nc.vector.tensor_tensor(sb[:, :], tr[:, :], bias_all[r0:r1, :], op=add)
            sqs = sq_pool.tile([GP, OUT_F], F32, tag="sqs")
            n2 = sq_pool.tile([GP, 1], F32, tag="n2")
            nc.vector.tensor_tensor(sqs[:, :], sb[:, :], sb[:, :], op=mult)
            nc.vector.tensor_reduce(
                n2[:, :], sqs[:, :], axis=mybir.AxisListType.X, op=add
            )
            rt = sq_pool.tile([GP, 1], F32, tag="rt")
            nc.scalar.activation(rt[:, :], n2[:, :], mybir.ActivationFunctionType.Sqrt)
            d1 = sq_pool.tile([GP, 1], F32, tag="d1")
            nc.vector.tensor_scalar_add(d1[:, :], n2[:, :], 1.0)
            d2 = sq_pool.tile([GP, 1], F32, tag="d2")
            nc.vector.tensor_scalar_add(d2[:, :], rt[:, :], EPS)
            den = sq_pool.tile([GP, 1], F32, tag="den")
            nc.vector.tensor_tensor(den[:, :], d1[:, :], d2[:, :], op=mult)
            rden = sq_pool.tile([GP, 1], F32, tag="rden")
            nc.vector.reciprocal(rden[:, :], den[:, :])
            coef = sq_pool.tile([GP, 1], F32, tag="coef")
            nc.vector.tensor_tensor(coef[:, :], n2[:, :], rden[:, :], op=mult)
            v = sq_pool.tile([GP, OUT_F], F32, tag="v")
            nc.vector.tensor_scalar_mul(v[:, :], sb[:, :], coef[:, 0:1])
            if is_final:
                out_rows = out_dram.ap().rearrange("b n o -> (b n) o")
                nc.sync.dma_start(out_rows[r0:r1, :], v[:, :])
                return None
            if V_prev is None:
                V_new = v
            else:
                V_new = sq_pool.tile([GP, OUT_F], F32, tag="V")
                nc.vector.tensor_tensor(V_new[:, :], V_prev[:, :], v[:, :], op=add)
            return V_new

        # ---- routing iterations (software-pipelined waves of WAVE pairs) ----
        V_cur = [None] * G
        s_cur = s_allT
        for k in range(2, routings + 1):
            s_next = sall_pool.tile([P, PAIRS], F32, tag="sall")
            for g in range(G):
                V_cur[g] = squash_group(s_cur, g, V_cur[g], is_final=False)
            nwaves = PAIRS // WAVE

            def make_wave(w0):
                g = w0 // GP
                gl0 = w0 % GP
                n0 = w0 % NUM
                NW = WAVE * OUT_F
                state = {}

                def s0():
                    # V rows -> flat row -> broadcast across partitions (PE)
                    V_flat = vflat_pool.tile([1, NW], F32, tag="vflat")
                    nc.sync.dma_start(V_flat[0:1, :], V_cur[g][gl0 : gl0 + WAVE, :])
                    vb_ps = psum_vb.tile([P, NW], F32, tag="vb")
                    for j in range(0, NW, 512):
                        jn = min(512, NW - j)
                        nc.tensor.matmul(
                            vb_ps[:, j : j + jn],
                            ones_row[:, :],
                            V_flat[0:1, j : j + jn],
                            start=True,
                            stop=True,
                        )
                    vb16 = small.tile([P, WAVE, OUT_F], BF16, tag="vb16")
                    nc.scalar.copy(vb16[:, :, :], vb_ps[:, :])
                    # u_hat for the whole wave in one op: W[n0:n0+W] * u (bcast o)
                    uh = wave_pool.tile([P, WAVE, T, OUT_F], BF16, tag="uhat")
                    for pl in range(WAVE):
                        p = w0 + pl
                        for t in range(T):
                            if (pl * T + t) % 6 == 0:
                                nc.scalar.mul(
                                    uh[:, pl, t, :],
                                    W_sb[:, n0 + pl, t, :],
                                    u_sb[:, t, p : p + 1],
                                )
                            else:
                                nc.gpsimd.tensor_scalar_mul(
                                    uh[:, pl, t, :],
                                    W_sb[:, n0 + pl, t, :],
                                    u_sb[:, t, p : p + 1],
                                )
                    state["vb16"] = vb16
                    state["uh"] = uh

                def s1():
                    # t = u_hat * V  (vb16 broadcast over chunks)
                    tt_ = wave_pool.tile([P, WAVE, T, OUT_F], BF16, tag="tt")
                    vb = state["vb16"][:, :, :]
                    vbb = bass.AP(
                        vb.tensor, vb.offset,
                        [vb.ap[0], [OUT_F, WAVE], [0, T], [1, OUT_F]],
                    )
                    nc.vector.tensor_tensor(
                        tt_[:, :, :, :], state["uh"][:, :, :, :], vbb, op=mult
                    )
                    state["tt"] = tt_

                def s2():
                    et = wave_pool.tile([P, WAVE, T, OUT_F], BF16, tag="e")
                    nc.scalar.activation(
                        et[:, :, :, :],
                        state["tt"][:, :, :, :],
                        mybir.ActivationFunctionType.Exp,
                    )
                    state["et"] = et

                def s3():
                    # Z[pair, chunk] = sum_o e;  uw = u / Z
                    Z = small.tile([P, WAVE, T], F32, tag="Z")
                    et = state["et"]
                    for pl in range(WAVE):
                        for t in range(T):
                            nc.vector.tensor_scalar(
                                et[:, pl, t, :],
                                et[:, pl, t, :],
                                1.0,
                                None,
                                mult,
                                op1=add,
                                accum_out=Z[:, pl, t : t + 1],
                            )
                    wr = small.tile([P, WAVE, T], F32, tag="wr")
                    nc.vector.reciprocal(wr[:, :, :], Z[:, :, :])
                    uw = small.tile([P, WAVE, T], BF16, tag="uw")
                    us = u_sb[:, :, w0 : w0 + WAVE]
                    usb = bass.AP(
                        us.tensor, us.offset, [us.ap[0], [1, WAVE], [PAIRS, T]]
                    )
                    nc.vector.tensor_tensor(uw[:, :, :], wr[:, :, :], usb, op=mult)
                    state["uw"] = uw

                def s4():
                    ft = wave_pool.tile([P, WAVE, T, OUT_F], BF16, tag="tt")
                    eng = nc.vector if (w0 // WAVE) % 7 < F_DVE_MOD else nc.gpsimd
                    eng.tensor_tensor(
                        ft[:, :, :, :],
                        state["et"][:, :, :, :],
                        W_sb[:, n0 : n0 + WAVE, :, :],
                        op=mult,
                    )
                    s_ps = psum_s.tile([OUT_F, WAVE], F32, tag="scol")
                    for pl in range(WAVE):
                        for t in range(T):
                            nc.tensor.matmul(
                                s_ps[:, pl : pl + 1],
                                ft[:, pl, t, :],
                                state["uw"][:, pl, t : t + 1],
                                start=(t == 0),
                                stop=(t == T - 1),
                            )
                    nc.scalar.copy(s_next[:, w0 : w0 + WAVE], s_ps[:, :])

                return [s0, s1, s2, s3, s4]

            waves = [make_wave(w * WAVE) for w in range(nwaves)]
            NSTAGE = 5
            for step in range(nwaves + NSTAGE - 1):
                for st in range(NSTAGE - 1, -1, -1):
                    w = step - st
                    if 0 <= w < nwaves:
                        waves[w][st]()
            s_cur = s_next

        for g in range(G):
            squash_group(s_cur, g, V_cur[g], is_final=True)

    nc.compile()
    return nc


_NC_CACHE = {}


def _get_nc(key):
    if key not in _NC_CACHE:
        _NC_CACHE[key] = _build(*key)
    return _NC_CACHE[key]


def _prep(u, weight, bias, c0, routings):
    u = np.ascontiguousarray(np.asarray(u, dtype=np.float32))
    weight = np.ascontiguousarray(
        np.asarray(weight, dtype=np.float32).reshape(weight.shape[-3:])
    )
    bias = np.ascontiguousarray(np.asarray(bias, dtype=np.float32).reshape(bias.shape[-2:]))
    c0 = np.ascontiguousarray(np.asarray(c0, dtype=np.float32).reshape(c0.shape[-2:]))
    routings = int(routings)
    B, NUM, IN_F = u.shape
    OUT_F = weight.shape[-1]
    uniform = bool(np.all(c0 == c0.flat[0]))
    c00 = float(c0.flat[0])
    assert B % N_CORES == 0, f"B={B} not divisible by {N_CORES}"
    B_core = B // N_CORES
    key = (B_core, NUM, IN_F, OUT_F, routings, c00 if uniform else 0.0, uniform)
    return u, weight, bias, c0, routings, B_core, key, uniform


def run_on_hw(u, weight, bias, c0, routings, trace=False):
    """Shard over cores, run SPMD, gather. Returns (out, exec_time_ns|None)."""
    u, weight, bias, c0, routings, B_core, key, uniform = _prep(
        u, weight, bias, c0, routings
    )
    nc = _get_nc(key)
    wbf = weight.astype(ml_dtypes.bfloat16)
    in_maps = []
    for c in range(N_CORES):
        m = {
            "u": u[c * B_core : (c + 1) * B_core],
            "wbf": wbf,
            "bias": bias,
        }
        if not uniform:
            m["c0"] = c0
        in_maps.append(m)
    res = run_bass_kernel_spmd(nc, in_maps, core_ids=list(range(N_CORES)), trace=trace)
    out = np.concatenate([res.results[c]["out"] for c in range(N_CORES)], axis=0)
    return out, res.exec_time_ns


_RUNNER_CACHE = {}


def _get_runner(key):
    """Cached jitted multi-core executable (avoids per-call re-jit)."""
    if key in _RUNNER_CACHE:
        return _RUNNER_CACHE[key]
    import jax
    from jax.sharding import Mesh, PartitionSpec
    from jax.experimental.shard_map import shard_map
    from concourse import bass2jax, mybir as mb

    nc = _get_nc(key)
    bass2jax.install_neuronx_cc_hook()
    part_name = nc.partition_id_tensor.name if nc.partition_id_tensor else None
    in_names, out_names, out_avals, zero_outs = [], [], [], []
    for alloc in nc.m.functions[0].allocations:
        if not isinstance(alloc, mb.MemoryLocationSet):
            continue
        name = alloc.memorylocations[0].name
        if alloc.kind == "ExternalInput":
            if name != part_name:
                in_names.append(name)
        elif alloc.kind == "ExternalOutput":
            out_names.append(name)
            shape = tuple(alloc.tensor_shape)
            dtype = mb.dt.np(alloc.dtype)
            out_avals.append(jax.core.ShapedArray(shape, dtype))
            zero_outs.append(np.zeros(shape, dtype))
    n_params = len(in_names)
    all_names = in_names + out_names
    if part_name is not None:
        all_names = all_names + [part_name]
    donate = tuple(range(n_params, n_params + len(out_names)))

    def _body(*args):
        operands = list(args)
        if part_name is not None:
            operands.append(bass2jax.partition_id_tensor())
        outs = bass2jax._bass_exec_p.bind(
            *operands,
            out_avals=tuple(out_avals),
            in_names=tuple(all_names),
            out_names=tuple(out_names),
            lowering_input_output_aliases=(),
            sim_require_finite=True,
            sim_require_nnan=True,
            nc=nc,
        )
        return tuple(outs)

    devices = jax.devices()[:N_CORES]
    mesh = Mesh(np.asarray(devices), ("core",))
    specs = (PartitionSpec("core"),) * (n_params + len(out_names))
    fn = jax.jit(
        shard_map(
            _body,
            mesh=mesh,
            in_specs=specs,
            out_specs=(PartitionSpec("core"),) * len(out_names),
            check_rep=False,
        ),
        donate_argnums=donate,
        keep_unused=True,
    )
    runner = (fn, in_names, out_names, out_avals, zero_outs)
    _RUNNER_CACHE[key] = runner
    return runner


def run_cached(u, weight, bias, c0, routings):
    """Run via a cached jitted executable. Returns (out, per_call_fn)."""
    u, weight, bias, c0, routings, B_core, key, uniform = _prep(
        u, weight, bias, c0, routings
    )
    fn, in_names, out_names, out_avals, zero_outs = _get_runner(key)
    wbf = weight.astype(ml_dtypes.bfloat16)
    per_core = {
        "u": [u[c * B_core : (c + 1) * B_core] for c in range(N_CORES)],
        "wbf": [wbf] * N_CORES,
        "bias": [bias] * N_CORES,
        "c0": [c0] * N_CORES,
    }
    concat_in = [np.concatenate(per_core[nm], axis=0) for nm in in_names]

    def call():
        zeros = [
            np.zeros((N_CORES * z.shape[0], *z.shape[1:]), z.dtype)
            for z in zero_outs
        ]
        outs = fn(*concat_in, *zeros)
        return np.asarray(outs[0])

    full = call()
    i = out_names.index("out")
    B_total = N_CORES * B_core
    out = full.reshape(N_CORES, B_core, *out_avals[i].shape[1:]).reshape(
        B_total, *out_avals[i].shape[1:]
    )
    return out, call


def kernel(**inputs):
    out, _ = run_on_hw(
        inputs["u"],
        inputs["weight"],
        inputs["bias"],
        inputs["c0"],
        inputs["routings"],
    )
    return out
